# revision 25
# baseline (speedup 1.0000x reference)
"""Trainium2 Bass kernel for nn_Block_17978733101066.

ConvNeXt-style block: channels-first LayerNorm -> NNMF conv (25 multiplicative
updates with grouped 3x3 convs) residual branch, then channels-last LayerNorm +
MLP residual branch.  Input x: (8, 96, 56, 56) f32.

Strategy: pure data parallel - one sample per NeuronCore (8 cores).  Per-core
layout keeps channels on SBUF partitions (C=96) and flattened spatial
positions on the free axis, chunked 448 wide.  Each grouped 3x3 conv is 9
PSUM-accumulated bf16 matmuls with per-offset block-diagonal (96x96) weight
matrices (host-built) against shifted views of a zero-padded (58x58) bf16
SBUF image.

Key scheduling/engine choices vs the naive version:
 - a ~5us dense warm-up matmul burst at kernel start keeps the PE HAM clock
   gate at 8/8 through the (otherwise sparse) LayerNorm prologue;
 - the per-chunk LayerNorm statistics rows are accumulated into adjacent
   PSUM partitions so the scalar follow-ups (mean/var/rsqrt) run once over a
   (7,448) tile instead of 7x over (1,448) rows;
 - channel-sum + broadcast is ONE matmul with an all-ones 96x96 stationary
   operand (out[o,p] = sum_c in[c,p] for every o), halving the PE overhead
   of the NNMF renormalisation;
 - all reciprocals run on the otherwise idle ACT engine (AF.Reciprocal with
   the eps as activation bias), keeping the DVE off the critical path;
 - x is DMA'd chunk-wise so the first LayerNorm chunk starts ~1.5us in.

Iteration 0's back-projection depends only on the constant h0 and ships as a
precomputed reciprocal.  All residual-path arithmetic stays f32.
"""

import numpy as np

C = 96
H = W = 56
NPIX = H * W          # 3136
HP = H + 2            # 58
PADPIX = HP * HP      # 3364
G, CG = 4, 24
NIT = 25
EPS = 1e-12
CH = 8                # image rows per chunk
NCHUNK = H // CH      # 7
CW = CH * W           # 448 positions per chunk
HID = 384

TRACE = False         # set True (e.g. from test.py) to collect NTFF exec time
LAST_RESULT = None    # BassKernelResults of the most recent run

_CACHED_NC = None


def _build_conv_mats(w_nnmf):
    """Per-offset lhsT matrices for both convs, packed (96, 9*96) f32."""
    w = np.abs(np.asarray(w_nnmf, np.float64))
    w = w / (w.sum(axis=(1, 2, 3), keepdims=True) + EPS)  # (96, 24, 3, 3)
    Wc = np.zeros((9, C, C), np.float64)  # [k, i, o] = w[o, i_loc, dy, dx]
    Wr = np.zeros((9, C, C), np.float64)  # [k, o, i] = w[o, i_loc, 2-dy, 2-dx]
    for dy in range(3):
        for dx in range(3):
            k = dy * 3 + dx
            blkc = w[:, :, dy, dx]          # (96 out, 24 in_local)
            blkr = w[:, :, 2 - dy, 2 - dx]  # (96 out, 24 in_local)
            for g in range(G):
                rows = slice(g * CG, (g + 1) * CG)
                Wc[k, rows, rows] = blkc[rows, :].T
                Wr[k, rows, rows] = blkr[rows, :]
    WcD = np.ascontiguousarray(Wc.transpose(1, 0, 2).reshape(C, 9 * C), np.float32)
    WrD = np.ascontiguousarray(Wr.transpose(1, 0, 2).reshape(C, 9 * C), np.float32)
    # iteration-0 back-projection is data independent (h0 is the constant
    # 1/C fill): ship 1/(convT(h0) + eps) as a precomputed input
    hpad0 = np.zeros((C, HP, HP))
    hpad0[:, 1:1 + H, 1:1 + W] = 1.0 / C
    recon0 = np.zeros((C, H * W))
    for dy in range(3):
        for dx in range(3):
            k = dy * 3 + dx
            view = hpad0[:, dy:dy + H, dx:dx + W].reshape(C, H * W)
            recon0 += Wr[k].T @ view
    rec0 = (1.0 / (recon0 + EPS)).astype(np.float32)
    return WcD, WrD, np.ascontiguousarray(rec0)


def _build_bass(nit=NIT, gelu_mode="hw"):
    import concourse.bass as bass
    import concourse.bacc as bacc
    import concourse.mybir as mybir
    from concourse.tile import TileContext

    f32 = mybir.dt.float32
    bf16 = mybir.dt.bfloat16
    AF = mybir.ActivationFunctionType
    OP = mybir.AluOpType

    nc = bacc.Bacc(None, target_bir_lowering=False)

    x_d = nc.declare_dram_parameter("x", [C, NPIX], f32, isOutput=False)
    rec0_d = nc.declare_dram_parameter("rec0", [C, NPIX], bf16, isOutput=False)
    wr_d = nc.declare_dram_parameter("wrecon", [C, 9 * C], bf16, isOutput=False)
    wc_d = nc.declare_dram_parameter("wconv", [C, 9 * C], bf16, isOutput=False)
    w1_d = nc.declare_dram_parameter("w1T", [C, HID], bf16, isOutput=False)
    b1_d = nc.declare_dram_parameter("b1", [HID, 1], f32, isOutput=False)
    w2_d = nc.declare_dram_parameter("w2T", [HID, C], bf16, isOutput=False)
    b2_d = nc.declare_dram_parameter("b2", [C, 1], f32, isOutput=False)
    ln1w_d = nc.declare_dram_parameter("ln1w", [C, 1], f32, isOutput=False)
    ln1b_d = nc.declare_dram_parameter("ln1b", [C, 1], f32, isOutput=False)
    sel1_d = nc.declare_dram_parameter("sel1", [C, NCHUNK * NCHUNK], bf16,
                                       isOutput=False)
    selrow_d = nc.declare_dram_parameter("selrow", [NCHUNK, NCHUNK * C], bf16,
                                         isOutput=False)
    out_d = nc.declare_dram_parameter("out", [C, NPIX], f32, isOutput=True)

    with TileContext(nc) as tc:
        with (
            tc.tile_pool(name="persist", bufs=1) as pp,
            tc.tile_pool(name="work", bufs=4) as wp,
            tc.tile_pool(name="small", bufs=3) as sp,
            tc.tile_pool(name="psconv", bufs=4, space="PSUM") as ps_conv,
            tc.tile_pool(name="psbig", bufs=2, space="PSUM") as ps_big,
            tc.tile_pool(name="psstat", bufs=1, space="PSUM") as ps_stat,
        ):
            # ---- persistent tiles ----
            xs = pp.tile([C, NPIX], f32, tag="xs")        # original x (residual)
            x2s = pp.tile([C, NPIX], f32, tag="x2s")      # x + attn residual
            xin = pp.tile([C, NPIX], bf16, tag="xin")     # normalized relu(LN1(x))
            rec0s = pp.tile([C, NPIX], bf16, tag="rec0s")  # 1/(convT(h0)+eps)
            hpad = pp.tile([C, PADPIX], bf16, tag="hpad")
            rpad = pp.tile([C, PADPIX], bf16, tag="rpad")
            wr = pp.tile([C, 9 * C], bf16, tag="wr")
            wc = pp.tile([C, 9 * C], bf16, tag="wc")
            w1s = pp.tile([C, HID], bf16, tag="w1s")
            w2s = [pp.tile([128, C], bf16, tag=f"w2s{k}", name=f"w2s{k}")
                   for k in range(3)]
            b1s = pp.tile([128, 3], f32, tag="b1s")
            b2s = pp.tile([C, 1], f32, tag="b2s")
            ln1w = pp.tile([C, 1], f32, tag="ln1w")
            ln1b = pp.tile([C, 1], f32, tag="ln1b")
            ones_sq = pp.tile([C, C], bf16, tag="ones_sq")     # sum+bcast lhsT
            # selector stationaries: sel1 block c = (96,7) with column c ones
            # (accumulate chunk c's channel-sum into psum partition c); selrow
            # block c = (7,96) with row c ones (broadcast stats row c to all
            # 96 partitions).
            sel1 = pp.tile([C, NCHUNK * NCHUNK], bf16, tag="sel1")
            selrow = pp.tile([NCHUNK, NCHUNK * C], bf16, tag="selrow")
            eps6 = pp.tile([16, 1], f32, tag="eps6")           # 1e-6 (LN1)
            eps5 = pp.tile([16, 1], f32, tag="eps5")           # 1e-5 (LN2)
            warm = pp.tile([128, 576], bf16, tag="warm")       # HAM warm-up junk
            # batched LayerNorm statistics rows (chunk c on partition c)
            ln_u = pp.tile([NCHUNK, CW], bf16, tag="ln_u")
            ln_isd = pp.tile([NCHUNK, CW], bf16, tag="ln_isd")
            ln_u2 = pp.tile([NCHUNK, CW], f32, tag="ln_u2")
            ln_var = pp.tile([NCHUNK, CW], f32, tag="ln_var")
            m_u = pp.tile([NCHUNK, CW], bf16, tag="m_u")
            m_isd = pp.tile([NCHUNK, CW], bf16, tag="m_isd")
            m_u2 = pp.tile([NCHUNK, CW], f32, tag="m_u2")
            m_var = pp.tile([NCHUNK, CW], f32, tag="m_var")

            # ---- HAM warm-up: ~5us of dense junk matmuls so the PE clock
            # gate opens before the LayerNorm prologue (otherwise the first
            # ~90us run at 1.2 GHz). Runs while the input DMAs land. ----
            nc.vector.memset(warm[:], 0.0)
            wps = ps_conv.tile([128, CW], f32, tag="conv", name="warmps")
            for _ in range(24):
                nc.tensor.matmul(wps[:], warm[:, 0:128], warm[:, 128:576])

            def fillers(n):
                # keep-warm junk matmuls into the (consumer-less) warm-up
                # psum tile: the LN/MLP phases leave the PE below the HAM
                # busy threshold, and once the clock gate closes every real
                # matmul runs at half rate.
                for _ in range(n):
                    nc.tensor.matmul(wps[:], warm[:, 0:128], warm[:, 128:576])

            # ---- load inputs. Sync queue: LN params + x chunks + conv
            # weights, ordered by first use. gpsimd queue takes the rest. ----
            nc.sync.dma_start(ln1w[:], ln1w_d[:])
            nc.sync.dma_start(ln1b[:], ln1b_d[:])
            nc.sync.dma_start(sel1[:], sel1_d[:])
            nc.sync.dma_start(selrow[:], selrow_d[:])
            nc.sync.dma_start(xs[:, 0:CW], x_d[:, 0:CW])
            nc.sync.dma_start(wc[:], wc_d[:])
            for c in range(1, NCHUNK):
                nc.sync.dma_start(xs[:, c * CW:(c + 1) * CW],
                                  x_d[:, c * CW:(c + 1) * CW])
            nc.sync.dma_start(wr[:], wr_d[:])
            nc.gpsimd.dma_start(rec0s[:], rec0_d[:])
            nc.gpsimd.dma_start(w1s[:], w1_d[:])
            for k in range(3):
                nc.gpsimd.dma_start(w2s[k][:], w2_d[k * 128:(k + 1) * 128, :])
            nc.gpsimd.dma_start(b1s[:],
                                b1_d[:].rearrange("(k p) one -> p (k one)", p=128))
            nc.gpsimd.dma_start(b2s[:], b2_d[:])

            nc.vector.memset(ones_sq[:], 1.0)
            nc.vector.memset(eps6[:], 1e-6)
            nc.vector.memset(eps5[:], 1e-5)

            def pad3(t):
                return t[:].rearrange("p (h w) -> p h w", h=HP)

            def interior(t, y0, nrows):
                return pad3(t)[0:C, 1 + y0:1 + y0 + nrows, 1:1 + W]

            # border-only init: the interiors are covered by the 1/C fill
            # (hpad) and by the ratio multiplies (rpad), so only the zero
            # borders need explicit memsets.
            for t in (hpad, rpad):
                nc.vector.memset(pad3(t)[0:C, 0:1, :], 0.0)        # top row
                nc.vector.memset(pad3(t)[0:C, HP - 1:HP, :], 0.0)  # bottom row
                nc.vector.memset(pad3(t)[0:C, 1:HP - 1, 0:1], 0.0)      # left
                nc.vector.memset(pad3(t)[0:C, 1:HP - 1, HP - 1:HP], 0.0)  # right
            nc.vector.memset(interior(hpad, 0, H), 1.0 / C)

            def sumbcast(src_ap, name="sb"):
                """ones 96x96 matmul: out[o,p] = sum_c src[c,p] for all o."""
                b = ps_big.tile([C, CW], f32, tag="bcast", name=name)
                nc.tensor.matmul(b[:], ones_sq[:], src_ap)
                return b

            def bcast_row(rows_t, c, name="bc"):
                """broadcast row c of a (7, CW) stats tile to all 96
                partitions: selector (7,96) lhsT with row c all-ones."""
                b = ps_big.tile([C, CW], f32, tag="bcast", name=name)
                nc.tensor.matmul(b[:], selrow[:, C * c:C * (c + 1)],
                                 rows_t[0:NCHUNK, :])
                return b

            # ---- LN stats, phase 1: chunk c's channel-sums of x and x^2
            # accumulate into psum partition c of two (7, CW) tiles (selector
            # stationary: column c ones). ----
            def stats_mms(pstatA, pstatB, c, xc_f32, xbc):
                sq = wp.tile([C, CW], bf16, tag="ln_sq")
                nc.scalar.square(sq[:], xc_f32)
                sel = sel1[:, NCHUNK * c:NCHUNK * (c + 1)]
                nc.tensor.matmul(pstatA[0:NCHUNK, :], sel, xbc,
                                 start=(c == 0), stop=(c == NCHUNK - 1))
                nc.tensor.matmul(pstatB[0:NCHUNK, :], sel, sq[:],
                                 start=(c == 0), stop=(c == NCHUNK - 1))

            # ---- LN stats, phase 2 (batched): mean/istd for all chunks ----
            def stats_rows(pstatA, pstatB, u_t, u2_t, var_t, isd_t, eps_t):
                nc.scalar.activation(u2_t[:], pstatA[0:NCHUNK, :], AF.Square,
                                     scale=1.0 / C)
                nc.vector.scalar_tensor_tensor(
                    var_t[:], pstatB[0:NCHUNK, :], 1.0 / C, u2_t[:],
                    OP.mult, OP.subtract)
                sd_t = sp.tile([NCHUNK, CW], f32, tag="sd")
                nc.scalar.activation(sd_t[:], var_t[:], AF.Sqrt,
                                     bias=eps_t[0:NCHUNK, 0:1])
                isdf_t = sp.tile([NCHUNK, CW], f32, tag="isdf")
                nc.vector.reciprocal_approx_fast(out=isdf_t[:], in_=sd_t[:])
                nc.vector.tensor_copy(isd_t[:], isdf_t[:])
                nc.scalar.activation(u_t[:], pstatA[0:NCHUNK, :], AF.Identity,
                                     scale=1.0 / C)

            # ---- LN1 per-chunk normalize + relu + channel-normalize ----
            def ln1_finish(c):
                sl = slice(c * CW, (c + 1) * CW)
                xc = xs[:, sl]
                ub = bcast_row(ln_u, c, name="ubc")
                ib = bcast_row(ln_isd, c, name="ibc")
                xm = wp.tile([C, CW], f32, tag="ln_xm")
                nc.vector.tensor_tensor(xm[:], xc, ub[:], OP.subtract)
                xn = wp.tile([C, CW], f32, tag="ln_xn")
                nc.vector.tensor_tensor(xn[:], xm[:], ib[:], OP.mult)
                rl = wp.tile([C, CW], bf16, tag="ln_rl")
                nc.scalar.activation(rl[:], xn[:], AF.Relu,
                                     bias=ln1b[:, 0:1], scale=ln1w[:, 0:1])
                sb = sumbcast(rl[:], name="lnsb")
                rb = wp.tile([C, CW], f32, tag="ln_rb")
                nc.vector.reciprocal_approx_fast(out=rb[:], in_=sb[:])
                nc.vector.tensor_tensor(xin[:, sl], rl[:], rb[:], OP.mult)

            # ---- NNMF phase builders ----
            def phase_a(c):
                # recon = convT(h); rec = 1/(recon+eps) in ONE ACT op
                y0 = c * CH
                ps = ps_conv.tile([C, CW], f32, tag="conv", name="psA")
                for k in range(9):
                    dy, dx = k // 3, k % 3
                    view = pad3(hpad)[0:C, y0 + dy:y0 + dy + CH, dx:dx + W]
                    nc.tensor.matmul(ps[:], wr[:, k * C:(k + 1) * C], view,
                                     start=(k == 0), stop=(k == 8))
                # recon >= ~1e-5 everywhere (strictly positive weights and h),
                # so the reference's +1e-12 guard is numerically irrelevant
                # and the reciprocal reads PSUM directly.
                rec = wp.tile([C, CW], f32, tag="rec", bufs=6)
                nc.vector.reciprocal_approx_fast(out=rec[:], in_=ps[:])
                ratio_mult(c, rec)

            def ratio_mult(c, rec):
                # ratio = xin * (1/recon), on gpsimd (latency hidden by skew)
                y0 = c * CH
                nc.gpsimd.tensor_tensor(
                    interior(rpad, y0, CH),
                    xin[:, c * CW:(c + 1) * CW], rec[:], OP.mult)

            def phase_b1(c):
                # conv(ratio) and ht = h * conv
                y0 = c * CH
                ps = ps_conv.tile([C, CW], f32, tag="conv", name="psB")
                for k in range(9):
                    dy, dx = k // 3, k % 3
                    view = pad3(rpad)[0:C, y0 + dy:y0 + dy + CH, dx:dx + W]
                    nc.tensor.matmul(ps[:], wc[:, k * C:(k + 1) * C], view,
                                     start=(k == 0), stop=(k == 8))
                ht = wp.tile([C, CW], bf16, tag="ht", bufs=6)
                nc.vector.tensor_tensor(ht[:], interior(hpad, y0, CH), ps[:],
                                        OP.mult)
                return ht

            def phase_b2(c, ht):
                # channel sum broadcast to every partition in one matmul
                # (ones 96x96 stationary), then 1/S on the DVE
                sb = sumbcast(ht[:], name="nsb")
                rb = wp.tile([C, CW], f32, tag="nrb", bufs=6)
                nc.vector.reciprocal_approx_fast(out=rb[:], in_=sb[:])
                return rb

            def phase_b3(c, ht, rb):
                y0 = c * CH
                nc.vector.tensor_tensor(interior(hpad, y0, CH), ht[:], rb[:],
                                        OP.mult)

            # ---- LN2 + MLP + residual ----
            def mlp_p1(pstatA, pstatB, c):
                sl = slice(c * CW, (c + 1) * CW)
                nc.gpsimd.tensor_tensor(x2s[:, sl], xs[:, sl],
                                        interior(hpad, c * CH, CH), OP.add)
                xc = x2s[:, sl]
                x2b = wp.tile([C, CW], bf16, tag="x2b")
                nc.vector.tensor_copy(x2b[:], xc)
                stats_mms(pstatA, pstatB, c, xc, x2b[:])

            def mlp_p2(c):
                sl = slice(c * CW, (c + 1) * CW)
                xc = x2s[:, sl]
                ub = bcast_row(m_u, c, name="ubc")
                ib = bcast_row(m_isd, c, name="ibc")
                xm = wp.tile([C, CW], f32, tag="ln_xm")
                nc.vector.tensor_tensor(xm[:], xc, ub[:], OP.subtract)
                # LN2's affine is folded into w1/b1 on the host, so the
                # normalized value feeds the matmul directly (as bf16).
                xn = wp.tile([C, CW], bf16, tag="ln_xw", bufs=8)
                nc.vector.tensor_tensor(xn[:], xm[:], ib[:], OP.mult)
                return xn

            def mlp_p3(c, xn):
                ys = []
                for j in range(3):
                    p1 = ps_big.tile([128, CW], f32, tag="bcast", name="p1")
                    nc.tensor.matmul(p1[:], w1s[:, j * 128:(j + 1) * 128], xn[:])
                    y1 = wp.tile([128, CW], bf16, tag=f"mlp_y{j}", name=f"mlp_y{j}")
                    if gelu_mode == "hw":
                        nc.scalar.activation(y1[:], p1[:], AF.Gelu,
                                             bias=b1s[:, j:j + 1])
                    else:
                        # CoreSim fallback: sigmoid-GELU (Gelu not implemented
                        # in the simulator). Mirror must match.
                        pre = wp.tile([128, CW], f32, tag=f"mlp_p{j}",
                                      name=f"mlp_p{j}")
                        nc.scalar.activation(pre[:], p1[:], AF.Identity,
                                             bias=b1s[:, j:j + 1])
                        sg = wp.tile([128, CW], f32, tag=f"mlp_s{j}",
                                     name=f"mlp_s{j}")
                        nc.scalar.activation(sg[:], pre[:], AF.Sigmoid,
                                             scale=1.702)
                        nc.vector.tensor_tensor(y1[:], pre[:], sg[:], OP.mult)
                    ys.append(y1)
                return ys

            def mlp_p4(c, ys):
                sl = slice(c * CW, (c + 1) * CW)
                p2 = ps_conv.tile([C, CW], f32, tag="conv")
                for k in range(3):
                    nc.tensor.matmul(p2[:], w2s[k][:], ys[k][:],
                                     start=(k == 0), stop=(k == 2))
                oc = wp.tile([C, CW], f32, tag="oc")
                nc.vector.scalar_tensor_tensor(
                    oc[:], p2[:], b2s[:, 0:1], x2s[:, sl], OP.add, OP.add)
                if c % 2 == 0:
                    nc.sync.dma_start(out_d[:, sl], oc[:])
                else:
                    nc.gpsimd.dma_start(out_d[:, sl], oc[:])

            # ---- LN1 stats phase (prologue, batched rows) ----
            pstat1A = ps_stat.tile([NCHUNK, CW], f32, tag="statA",
                                   name="pstat1A")
            pstat1B = ps_stat.tile([NCHUNK, CW], f32, tag="statB",
                                   name="pstat1B")
            for c in range(NCHUNK):
                sl = slice(c * CW, (c + 1) * CW)
                xbc = wp.tile([C, CW], bf16, tag="x2b")
                nc.vector.tensor_copy(xbc[:], xs[:, sl])
                stats_mms(pstat1A, pstat1B, c, xs[:, sl], xbc[:])
                fillers(3)
            stats_rows(pstat1A, pstat1B, ln_u, ln_u2, ln_var, ln_isd, eps6)
            fillers(6)

            # ---- ONE global software pipeline: LN1-finish chunks play the
            # A-stage role for iteration 0 (its back-projection reciprocal is
            # the precomputed rec0), then every NNMF chunk-slot, then the MLP
            # stages ride the tail. ----
            total = nit * NCHUNK
            hts = {}
            rbs = {}
            xns = {}
            yss = {}
            pstat2A = pstat2B = None
            for s in range(0, total + NCHUNK + 11):
                if s < min(NCHUNK, total):
                    ln1_finish(s)
                    ratio_mult(s, rec0s[:, s * CW:(s + 1) * CW])
                    fillers(5)
                elif s < total:
                    phase_a(s % NCHUNK)
                if total <= s:
                    # MLP tail slots are matmul-sparse; hold the clock open
                    fillers(4)
                c1 = s - 2
                if 0 <= c1 < total:
                    hts[c1] = phase_b1(c1 % NCHUNK)
                c2 = s - 3
                if 0 <= c2 < total:
                    rbs[c2] = phase_b2(c2 % NCHUNK, hts[c2])
                c3 = s - 4
                if 0 <= c3 < total:
                    phase_b3(c3 % NCHUNK, hts.pop(c3), rbs.pop(c3))
                # MLP stats (Square on ACT, sums on PE) trail the last
                # iteration's b3 slots; the batched row stage runs once after
                # all 7 chunks' sums are in.
                m1 = s - (total - 2)
                if 0 <= m1 < NCHUNK:
                    if pstat2A is None:
                        pstat2A = ps_stat.tile([NCHUNK, CW], f32,
                                               tag="statA", name="pstat2A")
                        pstat2B = ps_stat.tile([NCHUNK, CW], f32,
                                               tag="statB", name="pstat2B")
                    mlp_p1(pstat2A, pstat2B, m1)
                if m1 == NCHUNK:
                    stats_rows(pstat2A, pstat2B, m_u, m_u2, m_var, m_isd, eps5)
                m2 = s - (total + NCHUNK - 2)
                if 0 <= m2 < NCHUNK:
                    xns[m2] = mlp_p2(m2)
                m3 = s - (total + NCHUNK - 1)
                if 0 <= m3 < NCHUNK:
                    yss[m3] = mlp_p3(m3, xns.pop(m3))
                m4 = s - (total + NCHUNK)
                if 0 <= m4 < NCHUNK:
                    mlp_p4(m4, yss.pop(m4))

    return nc


def _prepare_maps(x, ln1_w, ln1_b, w_nnmf, ln2_w, ln2_b, w1, b1, w2, b2):
    import ml_dtypes
    bf16 = ml_dtypes.bfloat16
    WcD, WrD, rec0 = _build_conv_mats(w_nnmf)
    f = lambda a: np.ascontiguousarray(np.asarray(a, np.float32))
    fb = lambda a: np.ascontiguousarray(np.asarray(a, np.float32).astype(bf16))
    # LN2's per-channel affine folded into the first MLP matmul:
    # (xn*w + b) @ w1 + b1 == xn @ (diag(w) @ w1) + (b1 + b @ w1)
    w1_64 = np.asarray(w1, np.float64)
    w1f = w1_64 * np.asarray(ln2_w, np.float64)[:, None]
    b1f = np.asarray(b1, np.float64) + np.asarray(ln2_b, np.float64) @ w1_64
    # selector stationaries (see _build_bass)
    sel1 = np.zeros((C, NCHUNK * NCHUNK), np.float32)
    selrow = np.zeros((NCHUNK, NCHUNK * C), np.float32)
    for c in range(NCHUNK):
        sel1[:, NCHUNK * c + c] = 1.0
        selrow[c, C * c:C * (c + 1)] = 1.0
    shared = {
        "sel1": fb(sel1),
        "selrow": fb(selrow),
        "rec0": fb(rec0),
        "wrecon": fb(WrD),
        "wconv": fb(WcD),
        "w1T": fb(w1f),
        "b1": f(b1f).reshape(HID, 1),
        "w2T": fb(w2),
        "b2": f(b2).reshape(C, 1),
        "ln1w": f(ln1_w).reshape(C, 1),
        "ln1b": f(ln1_b).reshape(C, 1),
    }
    xs = np.asarray(x)
    return [dict(shared, x=f(xs[i]).reshape(C, NPIX))
            for i in range(xs.shape[0])]


def kernel(x, ln1_w, ln1_b, w_nnmf, ln2_w, ln2_b, w1, b1, w2, b2):
    global _CACHED_NC, LAST_RESULT
    from concourse.bass_utils import run_bass_kernel_spmd

    if _CACHED_NC is None:
        nc = _build_bass()
        nc.finalize()
        _CACHED_NC = nc
    nc = _CACHED_NC
    in_maps = _prepare_maps(x, ln1_w, ln1_b, w_nnmf, ln2_w, ln2_b, w1, b1, w2, b2)
    res = run_bass_kernel_spmd(nc, in_maps, core_ids=list(range(8)), trace=TRACE)
    LAST_RESULT = res
    out = np.stack([res.results[i]["out"].reshape(C, H, W) for i in range(8)])
    return out.astype(np.float32)


# revision 27
# speedup vs baseline: 1.1983x; 1.1983x over previous
"""Trainium2 Bass kernel for nn_Block_17978733101066.

ConvNeXt-style block: channels-first LayerNorm -> NNMF conv (25 multiplicative
updates with grouped 3x3 convs) residual branch, then channels-last LayerNorm +
MLP residual branch.  Input x: (8, 96, 56, 56) f32.

Strategy: pure data parallel - one sample per NeuronCore (8 cores).  Per-core
layout keeps channels on SBUF partitions (C=96) and flattened spatial
positions on the free axis, chunked 448 wide.  Each grouped 3x3 conv is 9
PSUM-accumulated bf16 matmuls with per-offset block-diagonal (96x96) weight
matrices (host-built) against shifted views of a zero-padded (58x58) bf16
SBUF image.

Key scheduling/engine choices vs the naive version:
 - a ~5us dense warm-up matmul burst at kernel start keeps the PE HAM clock
   gate at 8/8 through the (otherwise sparse) LayerNorm prologue;
 - the per-chunk LayerNorm statistics rows are accumulated into adjacent
   PSUM partitions so the scalar follow-ups (mean/var/rsqrt) run once over a
   (7,448) tile instead of 7x over (1,448) rows;
 - channel-sum + broadcast is ONE matmul with an all-ones 96x96 stationary
   operand (out[o,p] = sum_c in[c,p] for every o), halving the PE overhead
   of the NNMF renormalisation;
 - all reciprocals run on the otherwise idle ACT engine (AF.Reciprocal with
   the eps as activation bias), keeping the DVE off the critical path;
 - x is DMA'd chunk-wise so the first LayerNorm chunk starts ~1.5us in.

Iteration 0's back-projection depends only on the constant h0 and ships as a
precomputed reciprocal.  All residual-path arithmetic stays f32.
"""

import numpy as np

C = 96
H = W = 56
NPIX = H * W          # 3136
HP = H + 2            # 58
PADPIX = HP * HP      # 3364
G, CG = 4, 24
NIT = 25
EPS = 1e-12
CH = 8                # image rows per chunk
NCHUNK = H // CH      # 7
CW = CH * W           # 448 positions per chunk
HID = 384

TRACE = False         # set True (e.g. from test.py) to collect NTFF exec time
LAST_RESULT = None    # BassKernelResults of the most recent run

_CACHED_NC = None


def _build_conv_mats(w_nnmf):
    """Per-offset lhsT matrices for both convs, packed (96, 9*96) f32."""
    w = np.abs(np.asarray(w_nnmf, np.float64))
    w = w / (w.sum(axis=(1, 2, 3), keepdims=True) + EPS)  # (96, 24, 3, 3)
    Wc = np.zeros((9, C, C), np.float64)  # [k, i, o] = w[o, i_loc, dy, dx]
    Wr = np.zeros((9, C, C), np.float64)  # [k, o, i] = w[o, i_loc, 2-dy, 2-dx]
    for dy in range(3):
        for dx in range(3):
            k = dy * 3 + dx
            blkc = w[:, :, dy, dx]          # (96 out, 24 in_local)
            blkr = w[:, :, 2 - dy, 2 - dx]  # (96 out, 24 in_local)
            for g in range(G):
                rows = slice(g * CG, (g + 1) * CG)
                Wc[k, rows, rows] = blkc[rows, :].T
                Wr[k, rows, rows] = blkr[rows, :]
    WcD = np.ascontiguousarray(Wc.transpose(1, 0, 2).reshape(C, 9 * C), np.float32)
    WrD = np.ascontiguousarray(Wr.transpose(1, 0, 2).reshape(C, 9 * C), np.float32)
    # iteration-0 back-projection is data independent (h0 is the constant
    # 1/C fill): ship 1/(convT(h0) + eps) as a precomputed input
    hpad0 = np.zeros((C, HP, HP))
    hpad0[:, 1:1 + H, 1:1 + W] = 1.0 / C
    recon0 = np.zeros((C, H * W))
    for dy in range(3):
        for dx in range(3):
            k = dy * 3 + dx
            view = hpad0[:, dy:dy + H, dx:dx + W].reshape(C, H * W)
            recon0 += Wr[k].T @ view
    rec0 = (1.0 / (recon0 + EPS)).astype(np.float32)
    return WcD, WrD, np.ascontiguousarray(rec0)


def _build_bass(nit=NIT, gelu_mode="hw"):
    import concourse.bass as bass
    import concourse.bacc as bacc
    import concourse.mybir as mybir
    from concourse.tile import TileContext

    f32 = mybir.dt.float32
    bf16 = mybir.dt.bfloat16
    AF = mybir.ActivationFunctionType
    OP = mybir.AluOpType

    nc = bacc.Bacc(None, target_bir_lowering=False)

    x_d = nc.declare_dram_parameter("x", [C, NPIX], f32, isOutput=False)
    rec0_d = nc.declare_dram_parameter("rec0", [C, NPIX], bf16, isOutput=False)
    wr_d = nc.declare_dram_parameter("wrecon", [C, 9 * C], bf16, isOutput=False)
    wc_d = nc.declare_dram_parameter("wconv", [C, 9 * C], bf16, isOutput=False)
    w1_d = nc.declare_dram_parameter("w1T", [C, HID], bf16, isOutput=False)
    b1_d = nc.declare_dram_parameter("b1", [HID, 1], f32, isOutput=False)
    w2_d = nc.declare_dram_parameter("w2T", [HID, C], bf16, isOutput=False)
    b2_d = nc.declare_dram_parameter("b2", [C, 1], f32, isOutput=False)
    ln1w_d = nc.declare_dram_parameter("ln1w", [C, 1], f32, isOutput=False)
    ln1b_d = nc.declare_dram_parameter("ln1b", [C, 1], f32, isOutput=False)
    sel1_d = nc.declare_dram_parameter("sel1", [C, NCHUNK * NCHUNK], bf16,
                                       isOutput=False)
    selrow_d = nc.declare_dram_parameter("selrow", [NCHUNK, NCHUNK * C], bf16,
                                         isOutput=False)
    out_d = nc.declare_dram_parameter("out", [C, NPIX], f32, isOutput=True)

    with TileContext(nc) as tc:
        with (
            tc.tile_pool(name="persist", bufs=1) as pp,
            tc.tile_pool(name="work", bufs=4) as wp,
            tc.tile_pool(name="small", bufs=3) as sp,
            tc.tile_pool(name="psconv", bufs=3, space="PSUM") as ps_conv,
            tc.tile_pool(name="psbig", bufs=2, space="PSUM") as ps_big,
            tc.tile_pool(name="psstat", bufs=1, space="PSUM") as ps_stat,
            tc.tile_pool(name="psfill", bufs=1, space="PSUM") as ps_fill,
        ):
            # ---- persistent tiles ----
            xs = pp.tile([C, NPIX], f32, tag="xs")        # original x (residual)
            x2s = pp.tile([C, NPIX], f32, tag="x2s")      # x + attn residual
            xin = pp.tile([C, NPIX], bf16, tag="xin")     # normalized relu(LN1(x))
            rec0s = pp.tile([C, NPIX], bf16, tag="rec0s")  # 1/(convT(h0)+eps)
            hpad = pp.tile([C, PADPIX], bf16, tag="hpad")
            rpad = pp.tile([C, PADPIX], bf16, tag="rpad")
            wr = pp.tile([C, 9 * C], bf16, tag="wr")
            wc = pp.tile([C, 9 * C], bf16, tag="wc")
            w1s = pp.tile([C, HID], bf16, tag="w1s")
            w2s = [pp.tile([128, C], bf16, tag=f"w2s{k}", name=f"w2s{k}")
                   for k in range(3)]
            b1s = pp.tile([128, 3], f32, tag="b1s")
            b2s = pp.tile([C, 1], f32, tag="b2s")
            ln1w = pp.tile([C, 1], f32, tag="ln1w")
            ln1b = pp.tile([C, 1], f32, tag="ln1b")
            ones_sq = pp.tile([C, C], bf16, tag="ones_sq")     # sum+bcast lhsT
            # selector stationaries: sel1 block c = (96,7) with column c ones
            # (accumulate chunk c's channel-sum into psum partition c); selrow
            # block c = (7,96) with row c ones (broadcast stats row c to all
            # 96 partitions).
            sel1 = pp.tile([C, NCHUNK * NCHUNK], bf16, tag="sel1")
            selrow = pp.tile([NCHUNK, NCHUNK * C], bf16, tag="selrow")
            eps6 = pp.tile([16, 1], f32, tag="eps6")           # 1e-6 (LN1)
            eps5 = pp.tile([16, 1], f32, tag="eps5")           # 1e-5 (LN2)
            warm = pp.tile([128, 576], bf16, tag="warm")       # HAM warm-up junk
            # batched LayerNorm statistics rows (chunk c on partition c)
            ln_u = pp.tile([NCHUNK, CW], bf16, tag="ln_u")
            ln_isd = pp.tile([NCHUNK, CW], bf16, tag="ln_isd")
            ln_u2 = pp.tile([NCHUNK, CW], f32, tag="ln_u2")
            ln_var = pp.tile([NCHUNK, CW], f32, tag="ln_var")
            m_u = pp.tile([NCHUNK, CW], bf16, tag="m_u")
            m_isd = pp.tile([NCHUNK, CW], bf16, tag="m_isd")
            m_u2 = pp.tile([NCHUNK, CW], f32, tag="m_u2")
            m_var = pp.tile([NCHUNK, CW], f32, tag="m_var")

            # ---- HAM warm-up: ~5us of dense junk matmuls so the PE clock
            # gate opens before the LayerNorm prologue (otherwise the first
            # ~90us run at 1.2 GHz). Runs while the input DMAs land. ----
            nc.vector.memset(warm[:], 0.0)
            wps = ps_fill.tile([128, CW], f32, tag="fill", name="warmps")
            for _ in range(24):
                nc.tensor.matmul(wps[:], warm[:, 0:128], warm[:, 128:576])

            def fillers(n):
                # keep-warm junk matmuls into the (consumer-less) warm-up
                # psum tile: the LN/MLP phases leave the PE below the HAM
                # busy threshold, and once the clock gate closes every real
                # matmul runs at half rate.
                for _ in range(n):
                    nc.tensor.matmul(wps[:], warm[:, 0:128], warm[:, 128:576])

            # ---- load inputs. Sync queue: LN params + x chunks + conv
            # weights, ordered by first use. gpsimd queue takes the rest. ----
            nc.sync.dma_start(ln1w[:], ln1w_d[:])
            nc.sync.dma_start(ln1b[:], ln1b_d[:])
            nc.sync.dma_start(sel1[:], sel1_d[:])
            nc.sync.dma_start(selrow[:], selrow_d[:])
            nc.sync.dma_start(xs[:, 0:CW], x_d[:, 0:CW])
            nc.sync.dma_start(wc[:], wc_d[:])
            for c in range(1, NCHUNK):
                nc.sync.dma_start(xs[:, c * CW:(c + 1) * CW],
                                  x_d[:, c * CW:(c + 1) * CW])
            nc.sync.dma_start(wr[:], wr_d[:])
            nc.gpsimd.dma_start(rec0s[:], rec0_d[:])
            nc.gpsimd.dma_start(w1s[:], w1_d[:])
            for k in range(3):
                nc.gpsimd.dma_start(w2s[k][:], w2_d[k * 128:(k + 1) * 128, :])
            nc.gpsimd.dma_start(b1s[:],
                                b1_d[:].rearrange("(k p) one -> p (k one)", p=128))
            nc.gpsimd.dma_start(b2s[:], b2_d[:])

            nc.vector.memset(ones_sq[:], 1.0)
            nc.vector.memset(eps6[:], 1e-6)
            nc.vector.memset(eps5[:], 1e-5)

            def pad3(t):
                return t[:].rearrange("p (h w) -> p h w", h=HP)

            def interior(t, y0, nrows):
                return pad3(t)[0:C, 1 + y0:1 + y0 + nrows, 1:1 + W]

            # border-only init: the interiors are covered by the 1/C fill
            # (hpad) and by the ratio multiplies (rpad), so only the zero
            # borders need explicit memsets.
            for t in (hpad, rpad):
                nc.vector.memset(pad3(t)[0:C, 0:1, :], 0.0)        # top row
                nc.vector.memset(pad3(t)[0:C, HP - 1:HP, :], 0.0)  # bottom row
                nc.vector.memset(pad3(t)[0:C, 1:HP - 1, 0:1], 0.0)      # left
                nc.vector.memset(pad3(t)[0:C, 1:HP - 1, HP - 1:HP], 0.0)  # right
            nc.vector.memset(interior(hpad, 0, H), 1.0 / C)

            def sumbcast(src_ap, name="sb"):
                """ones 96x96 matmul: out[o,p] = sum_c src[c,p] for all o."""
                b = ps_big.tile([C, CW], f32, tag="bcast", name=name)
                nc.tensor.matmul(b[:], ones_sq[:], src_ap)
                return b

            def bcast_row(rows_t, c, name="bc"):
                """broadcast row c of a (7, CW) stats tile to all 96
                partitions: selector (7,96) lhsT with row c all-ones."""
                b = ps_big.tile([C, CW], f32, tag="bcast", name=name)
                nc.tensor.matmul(b[:], selrow[:, C * c:C * (c + 1)],
                                 rows_t[0:NCHUNK, :])
                return b

            # ---- LN stats, phase 1: chunk c's channel-sums of x and x^2
            # accumulate into psum partition c of two (7, CW) tiles (selector
            # stationary: column c ones). ----
            def stats_mms(pstatA, pstatB, c, xc_f32, xbc):
                sq = wp.tile([C, CW], bf16, tag="ln_sq")
                nc.scalar.square(sq[:], xc_f32)
                sel = sel1[:, NCHUNK * c:NCHUNK * (c + 1)]
                nc.tensor.matmul(pstatA[0:NCHUNK, :], sel, xbc,
                                 start=(c == 0), stop=(c == NCHUNK - 1))
                nc.tensor.matmul(pstatB[0:NCHUNK, :], sel, sq[:],
                                 start=(c == 0), stop=(c == NCHUNK - 1))

            # ---- LN stats, phase 2 (batched): mean/istd for all chunks ----
            def stats_rows(pstatA, pstatB, u_t, u2_t, var_t, isd_t, eps_t):
                nc.scalar.activation(u2_t[:], pstatA[0:NCHUNK, :], AF.Square,
                                     scale=1.0 / C)
                nc.vector.scalar_tensor_tensor(
                    var_t[:], pstatB[0:NCHUNK, :], 1.0 / C, u2_t[:],
                    OP.mult, OP.subtract)
                sd_t = sp.tile([NCHUNK, CW], f32, tag="sd")
                nc.scalar.activation(sd_t[:], var_t[:], AF.Sqrt,
                                     bias=eps_t[0:NCHUNK, 0:1])
                isdf_t = sp.tile([NCHUNK, CW], f32, tag="isdf")
                nc.vector.reciprocal_approx_fast(out=isdf_t[:], in_=sd_t[:])
                nc.vector.tensor_copy(isd_t[:], isdf_t[:])
                nc.scalar.activation(u_t[:], pstatA[0:NCHUNK, :], AF.Identity,
                                     scale=1.0 / C)

            # ---- LN1 per-chunk normalize + relu + channel-normalize ----
            def ln1_finish(c):
                sl = slice(c * CW, (c + 1) * CW)
                xc = xs[:, sl]
                ub = bcast_row(ln_u, c, name="ubc")
                ib = bcast_row(ln_isd, c, name="ibc")
                xm = wp.tile([C, CW], f32, tag="ln_xm")
                nc.vector.tensor_tensor(xm[:], xc, ub[:], OP.subtract)
                xn = wp.tile([C, CW], f32, tag="ln_xn")
                nc.vector.tensor_tensor(xn[:], xm[:], ib[:], OP.mult)
                rl = wp.tile([C, CW], bf16, tag="ln_rl")
                nc.scalar.activation(rl[:], xn[:], AF.Relu,
                                     bias=ln1b[:, 0:1], scale=ln1w[:, 0:1])
                sb = sumbcast(rl[:], name="lnsb")
                rb = wp.tile([C, CW], f32, tag="ln_rb")
                nc.vector.reciprocal_approx_fast(out=rb[:], in_=sb[:])
                nc.vector.tensor_tensor(xin[:, sl], rl[:], rb[:], OP.mult)

            # ---- NNMF phase builders ----
            def phase_a(c):
                # recon = convT(h); rec = 1/(recon+eps) in ONE ACT op
                y0 = c * CH
                ps = ps_conv.tile([C, CW], f32, tag="conv", name="psA")
                for k in range(9):
                    dy, dx = k // 3, k % 3
                    view = pad3(hpad)[0:C, y0 + dy:y0 + dy + CH, dx:dx + W]
                    nc.tensor.matmul(ps[:], wr[:, k * C:(k + 1) * C], view,
                                     start=(k == 0), stop=(k == 8))
                # recon >= ~1e-5 everywhere (strictly positive weights and h),
                # so the reference's +1e-12 guard is numerically irrelevant
                # and the reciprocal reads PSUM directly.
                rec = wp.tile([C, CW], f32, tag="rec", bufs=6)
                nc.vector.reciprocal_approx_fast(out=rec[:], in_=ps[:])
                ratio_mult(c, rec)

            def ratio_mult(c, rec):
                # ratio = xin * (1/recon), on gpsimd (latency hidden by skew)
                y0 = c * CH
                nc.gpsimd.tensor_tensor(
                    interior(rpad, y0, CH),
                    xin[:, c * CW:(c + 1) * CW], rec[:], OP.mult)

            def phase_b1(c):
                # conv(ratio) and ht = h * conv
                y0 = c * CH
                ps = ps_conv.tile([C, CW], f32, tag="conv", name="psB")
                for k in range(9):
                    dy, dx = k // 3, k % 3
                    view = pad3(rpad)[0:C, y0 + dy:y0 + dy + CH, dx:dx + W]
                    nc.tensor.matmul(ps[:], wc[:, k * C:(k + 1) * C], view,
                                     start=(k == 0), stop=(k == 8))
                ht = wp.tile([C, CW], bf16, tag="ht", bufs=6)
                nc.vector.tensor_tensor(ht[:], interior(hpad, y0, CH), ps[:],
                                        OP.mult)
                return ht

            def phase_b2(c, ht):
                # channel sum broadcast to every partition in one matmul
                # (ones 96x96 stationary), then 1/S on the DVE
                sb = sumbcast(ht[:], name="nsb")
                rb = wp.tile([C, CW], f32, tag="nrb", bufs=6)
                nc.vector.reciprocal_approx_fast(out=rb[:], in_=sb[:])
                return rb

            def phase_b3(c, ht, rb):
                y0 = c * CH
                nc.vector.tensor_tensor(interior(hpad, y0, CH), ht[:], rb[:],
                                        OP.mult)

            # ---- LN2 + MLP + residual ----
            def mlp_p1(pstatA, pstatB, c):
                sl = slice(c * CW, (c + 1) * CW)
                nc.gpsimd.tensor_tensor(x2s[:, sl], xs[:, sl],
                                        interior(hpad, c * CH, CH), OP.add)
                xc = x2s[:, sl]
                x2b = wp.tile([C, CW], bf16, tag="x2b")
                nc.vector.tensor_copy(x2b[:], xc)
                stats_mms(pstatA, pstatB, c, xc, x2b[:])

            def mlp_p2(c):
                sl = slice(c * CW, (c + 1) * CW)
                xc = x2s[:, sl]
                ub = bcast_row(m_u, c, name="ubc")
                ib = bcast_row(m_isd, c, name="ibc")
                xm = wp.tile([C, CW], f32, tag="ln_xm")
                nc.vector.tensor_tensor(xm[:], xc, ub[:], OP.subtract)
                # LN2's affine is folded into w1/b1 on the host, so the
                # normalized value feeds the matmul directly (as bf16).
                xn = wp.tile([C, CW], bf16, tag="ln_xw", bufs=8)
                nc.vector.tensor_tensor(xn[:], xm[:], ib[:], OP.mult)
                return xn

            def mlp_p3(c, xn):
                ys = []
                for j in range(3):
                    p1 = ps_big.tile([128, CW], f32, tag="bcast", name="p1")
                    nc.tensor.matmul(p1[:], w1s[:, j * 128:(j + 1) * 128], xn[:])
                    y1 = wp.tile([128, CW], bf16, tag=f"mlp_y{j}", name=f"mlp_y{j}")
                    if gelu_mode == "hw":
                        nc.scalar.activation(y1[:], p1[:], AF.Gelu,
                                             bias=b1s[:, j:j + 1])
                    else:
                        # CoreSim fallback: sigmoid-GELU (Gelu not implemented
                        # in the simulator). Mirror must match.
                        pre = wp.tile([128, CW], f32, tag=f"mlp_p{j}",
                                      name=f"mlp_p{j}")
                        nc.scalar.activation(pre[:], p1[:], AF.Identity,
                                             bias=b1s[:, j:j + 1])
                        sg = wp.tile([128, CW], f32, tag=f"mlp_s{j}",
                                     name=f"mlp_s{j}")
                        nc.scalar.activation(sg[:], pre[:], AF.Sigmoid,
                                             scale=1.702)
                        nc.vector.tensor_tensor(y1[:], pre[:], sg[:], OP.mult)
                    ys.append(y1)
                return ys

            def mlp_p4(c, ys):
                sl = slice(c * CW, (c + 1) * CW)
                p2 = ps_conv.tile([C, CW], f32, tag="conv")
                for k in range(3):
                    nc.tensor.matmul(p2[:], w2s[k][:], ys[k][:],
                                     start=(k == 0), stop=(k == 2))
                oc = wp.tile([C, CW], f32, tag="oc")
                nc.vector.scalar_tensor_tensor(
                    oc[:], p2[:], b2s[:, 0:1], x2s[:, sl], OP.add, OP.add)
                if c % 2 == 0:
                    nc.sync.dma_start(out_d[:, sl], oc[:])
                else:
                    nc.gpsimd.dma_start(out_d[:, sl], oc[:])

            # ---- LN1 stats phase (prologue, batched rows) ----
            pstat1A = ps_stat.tile([NCHUNK, CW], f32, tag="statA",
                                   name="pstat1A")
            pstat1B = ps_stat.tile([NCHUNK, CW], f32, tag="statB",
                                   name="pstat1B")
            for c in range(NCHUNK):
                sl = slice(c * CW, (c + 1) * CW)
                xbc = wp.tile([C, CW], bf16, tag="x2b")
                nc.vector.tensor_copy(xbc[:], xs[:, sl])
                stats_mms(pstat1A, pstat1B, c, xs[:, sl], xbc[:])
                fillers(3)
            stats_rows(pstat1A, pstat1B, ln_u, ln_u2, ln_var, ln_isd, eps6)
            fillers(6)

            # ---- ONE global software pipeline: LN1-finish chunks play the
            # A-stage role for iteration 0 (its back-projection reciprocal is
            # the precomputed rec0), then every NNMF chunk-slot, then the MLP
            # stages ride the tail. ----
            total = nit * NCHUNK
            hts = {}
            rbs = {}
            xns = {}
            yss = {}
            pstat2A = pstat2B = None
            for s in range(0, total + NCHUNK + 11):
                if s < min(NCHUNK, total):
                    ln1_finish(s)
                    ratio_mult(s, rec0s[:, s * CW:(s + 1) * CW])
                    fillers(5)
                elif s < total:
                    phase_a(s % NCHUNK)
                if total <= s:
                    # MLP tail slots are matmul-sparse; hold the clock open
                    fillers(4)
                c1 = s - 2
                if 0 <= c1 < total:
                    hts[c1] = phase_b1(c1 % NCHUNK)
                c2 = s - 3
                if 0 <= c2 < total:
                    rbs[c2] = phase_b2(c2 % NCHUNK, hts[c2])
                c3 = s - 4
                if 0 <= c3 < total:
                    phase_b3(c3 % NCHUNK, hts.pop(c3), rbs.pop(c3))
                # MLP stats (Square on ACT, sums on PE) trail the last
                # iteration's b3 slots; the batched row stage runs once after
                # all 7 chunks' sums are in.
                m1 = s - (total - 2)
                if 0 <= m1 < NCHUNK:
                    if pstat2A is None:
                        pstat2A = ps_stat.tile([NCHUNK, CW], f32,
                                               tag="statA", name="pstat2A")
                        pstat2B = ps_stat.tile([NCHUNK, CW], f32,
                                               tag="statB", name="pstat2B")
                    mlp_p1(pstat2A, pstat2B, m1)
                if m1 == NCHUNK:
                    stats_rows(pstat2A, pstat2B, m_u, m_u2, m_var, m_isd, eps5)
                m2 = s - (total + NCHUNK - 2)
                if 0 <= m2 < NCHUNK:
                    xns[m2] = mlp_p2(m2)
                m3 = s - (total + NCHUNK - 1)
                if 0 <= m3 < NCHUNK:
                    yss[m3] = mlp_p3(m3, xns.pop(m3))
                m4 = s - (total + NCHUNK)
                if 0 <= m4 < NCHUNK:
                    mlp_p4(m4, yss.pop(m4))

    return nc


def _prepare_maps(x, ln1_w, ln1_b, w_nnmf, ln2_w, ln2_b, w1, b1, w2, b2):
    import ml_dtypes
    bf16 = ml_dtypes.bfloat16
    WcD, WrD, rec0 = _build_conv_mats(w_nnmf)
    f = lambda a: np.ascontiguousarray(np.asarray(a, np.float32))
    fb = lambda a: np.ascontiguousarray(np.asarray(a, np.float32).astype(bf16))
    # LN2's per-channel affine folded into the first MLP matmul:
    # (xn*w + b) @ w1 + b1 == xn @ (diag(w) @ w1) + (b1 + b @ w1)
    w1_64 = np.asarray(w1, np.float64)
    w1f = w1_64 * np.asarray(ln2_w, np.float64)[:, None]
    b1f = np.asarray(b1, np.float64) + np.asarray(ln2_b, np.float64) @ w1_64
    # selector stationaries (see _build_bass)
    sel1 = np.zeros((C, NCHUNK * NCHUNK), np.float32)
    selrow = np.zeros((NCHUNK, NCHUNK * C), np.float32)
    for c in range(NCHUNK):
        sel1[:, NCHUNK * c + c] = 1.0
        selrow[c, C * c:C * (c + 1)] = 1.0
    shared = {
        "sel1": fb(sel1),
        "selrow": fb(selrow),
        "rec0": fb(rec0),
        "wrecon": fb(WrD),
        "wconv": fb(WcD),
        "w1T": fb(w1f),
        "b1": f(b1f).reshape(HID, 1),
        "w2T": fb(w2),
        "b2": f(b2).reshape(C, 1),
        "ln1w": f(ln1_w).reshape(C, 1),
        "ln1b": f(ln1_b).reshape(C, 1),
    }
    xs = np.asarray(x)
    return [dict(shared, x=f(xs[i]).reshape(C, NPIX))
            for i in range(xs.shape[0])]


def kernel(x, ln1_w, ln1_b, w_nnmf, ln2_w, ln2_b, w1, b1, w2, b2):
    global _CACHED_NC, LAST_RESULT
    from concourse.bass_utils import run_bass_kernel_spmd

    if _CACHED_NC is None:
        nc = _build_bass()
        nc.finalize()
        _CACHED_NC = nc
    nc = _CACHED_NC
    in_maps = _prepare_maps(x, ln1_w, ln1_b, w_nnmf, ln2_w, ln2_b, w1, b1, w2, b2)
    res = run_bass_kernel_spmd(nc, in_maps, core_ids=list(range(8)), trace=TRACE)
    LAST_RESULT = res
    out = np.stack([res.results[i]["out"].reshape(C, H, W) for i in range(8)])
    return out.astype(np.float32)


# revision 39
# speedup vs baseline: 1.3593x; 1.1344x over previous
"""Trainium2 Bass kernel for nn_Block_17978733101066.

ConvNeXt-style block: channels-first LayerNorm -> NNMF conv (25 multiplicative
updates with grouped 3x3 convs) residual branch, then channels-last LayerNorm +
MLP residual branch.  Input x: (8, 96, 56, 56) f32.

Strategy: pure data parallel - one sample per NeuronCore (8 cores).  Per-core
layout keeps channels on SBUF partitions (C=96) and flattened spatial
positions on the free axis, chunked 448 wide.  Each grouped 3x3 conv is 9
PSUM-accumulated bf16 matmuls with per-offset block-diagonal (96x96) weight
matrices (host-built) against shifted views of a zero-padded (58x58) bf16
SBUF image.

Key scheduling/engine choices vs the naive version:
 - a ~5us dense warm-up matmul burst at kernel start keeps the PE HAM clock
   gate at 8/8 through the (otherwise sparse) LayerNorm prologue;
 - the per-chunk LayerNorm statistics rows are accumulated into adjacent
   PSUM partitions so the scalar follow-ups (mean/var/rsqrt) run once over a
   (7,448) tile instead of 7x over (1,448) rows;
 - channel-sum + broadcast is ONE matmul with an all-ones 96x96 stationary
   operand (out[o,p] = sum_c in[c,p] for every o), halving the PE overhead
   of the NNMF renormalisation;
 - all reciprocals run on the otherwise idle ACT engine (AF.Reciprocal with
   the eps as activation bias), keeping the DVE off the critical path;
 - x is DMA'd chunk-wise so the first LayerNorm chunk starts ~1.5us in.

Iteration 0's back-projection depends only on the constant h0 and ships as a
precomputed reciprocal.  All residual-path arithmetic stays f32.
"""

import numpy as np

C = 96
H = W = 56
NPIX = H * W          # 3136
HP = H + 2            # 58
PADPIX = HP * HP      # 3364
WP8 = 64              # fp8 padded-image row stride (16B-aligned pairs)
PADPIX8 = HP * WP8    # 3712
G, CG = 4, 24
NIT = 25
EPS = 1e-12
CH = 8                # image rows per chunk
NCHUNK = H // CH      # 7
CW = CH * W           # 448 positions per chunk
HID = 384
# fp8 scale plan: hpad stores 64*h, recon weights store 512*wr, the ones
# sum+bcast matrix stores 1/8192 (so xin carries 8192x and the normalizer
# reciprocals absorb the rest); conv weights store 4*wc so psB is the true
# conv(ratio).
SH, SW, QS = 64.0, 512.0, 8192.0

TRACE = False         # set True (e.g. from test.py) to collect NTFF exec time
LAST_RESULT = None    # BassKernelResults of the most recent run

_CACHED_NC = None


def _build_conv_mats(w_nnmf):
    """Per-offset lhsT matrices for both convs, packed (96, 9*96) f32."""
    w = np.abs(np.asarray(w_nnmf, np.float64))
    w = w / (w.sum(axis=(1, 2, 3), keepdims=True) + EPS)  # (96, 24, 3, 3)
    Wc = np.zeros((9, C, C), np.float64)  # [k, i, o] = w[o, i_loc, dy, dx]
    Wr = np.zeros((9, C, C), np.float64)  # [k, o, i] = w[o, i_loc, 2-dy, 2-dx]
    for dy in range(3):
        for dx in range(3):
            k = dy * 3 + dx
            blkc = w[:, :, dy, dx]          # (96 out, 24 in_local)
            blkr = w[:, :, 2 - dy, 2 - dx]  # (96 out, 24 in_local)
            for g in range(G):
                rows = slice(g * CG, (g + 1) * CG)
                Wc[k, rows, rows] = blkc[rows, :].T
                Wr[k, rows, rows] = blkr[rows, :]
    # conv(ratio) stationaries: x4 so psB is conv(ratio_true) given the
    # stored ratio is ratio/4
    WcD = np.ascontiguousarray(
        4.0 * Wc.transpose(1, 0, 2).reshape(C, 9 * C), np.float32)
    # recon stationaries in fp8 DoubleRow layout: three (dy0|dy1) pair
    # blocks (one per dx), then the three dy=2 singles; values x512 to
    # clear the fp8e4 denormal range
    Wr8 = np.zeros((C, 9 * C), np.float64)
    for dx in range(3):
        Wr8[:, dx * 2 * C:dx * 2 * C + C] = Wr[0 * 3 + dx]
        Wr8[:, dx * 2 * C + C:(dx + 1) * 2 * C] = Wr[1 * 3 + dx]
        Wr8[:, 6 * C + dx * C:6 * C + (dx + 1) * C] = Wr[2 * 3 + dx]
    Wr8 = np.ascontiguousarray(SW * Wr8, np.float32)
    # iteration-0 back-projection is data independent (h0 is the constant
    # 1/C fill): ship 1/(SH*SW*(convT(h0) + eps)) as a precomputed input
    hpad0 = np.zeros((C, HP, HP))
    hpad0[:, 1:1 + H, 1:1 + W] = 1.0 / C
    recon0 = np.zeros((C, H * W))
    for dy in range(3):
        for dx in range(3):
            k = dy * 3 + dx
            view = hpad0[:, dy:dy + H, dx:dx + W].reshape(C, H * W)
            recon0 += Wr[k].T @ view
    rec0 = (1.0 / (SH * SW * (recon0 + EPS))).astype(np.float32)
    return WcD, Wr8, np.ascontiguousarray(rec0)


def _build_bass(nit=NIT, gelu_mode="hw"):
    import concourse.bass as bass
    import concourse.bacc as bacc
    import concourse.mybir as mybir
    from concourse.tile import TileContext

    f32 = mybir.dt.float32
    bf16 = mybir.dt.bfloat16
    fp8 = mybir.dt.float8e4
    AF = mybir.ActivationFunctionType
    OP = mybir.AluOpType
    DR = mybir.MatmulPerfMode.DoubleRow

    nc = bacc.Bacc(None, target_bir_lowering=False)

    x_d = nc.declare_dram_parameter("x", [C, NPIX], f32, isOutput=False)
    rec0_d = nc.declare_dram_parameter("rec0", [C, NPIX], bf16, isOutput=False)
    wr_d = nc.declare_dram_parameter("wrecon", [C, 9 * C], fp8, isOutput=False)
    wc_d = nc.declare_dram_parameter("wconv", [C, 9 * C], bf16, isOutput=False)
    w1_d = nc.declare_dram_parameter("w1T", [C, HID], bf16, isOutput=False)
    b1_d = nc.declare_dram_parameter("b1", [HID, 1], f32, isOutput=False)
    w2_d = nc.declare_dram_parameter("w2T", [HID, C], bf16, isOutput=False)
    b2_d = nc.declare_dram_parameter("b2", [C, 1], f32, isOutput=False)
    ln1w_d = nc.declare_dram_parameter("ln1w", [C, 1], f32, isOutput=False)
    ln1b_d = nc.declare_dram_parameter("ln1b", [C, 1], f32, isOutput=False)
    sel1_d = nc.declare_dram_parameter("sel1", [C, NCHUNK * NCHUNK], bf16,
                                       isOutput=False)
    selrow_d = nc.declare_dram_parameter("selrow", [NCHUNK, NCHUNK * C], bf16,
                                         isOutput=False)
    out_d = nc.declare_dram_parameter("out", [C, NPIX], f32, isOutput=True)

    with TileContext(nc) as tc:
        with (
            tc.tile_pool(name="persist", bufs=1) as pp,
            tc.tile_pool(name="work", bufs=4) as wp,
            tc.tile_pool(name="small", bufs=3) as sp,
            tc.tile_pool(name="psconv", bufs=3, space="PSUM") as ps_conv,
            tc.tile_pool(name="psbig", bufs=2, space="PSUM") as ps_big,
            tc.tile_pool(name="psstat", bufs=1, space="PSUM") as ps_stat,
            tc.tile_pool(name="psfill", bufs=1, space="PSUM") as ps_fill,
        ):
            # ---- persistent tiles ----
            xs = pp.tile([C, NPIX], f32, tag="xs")        # original x (residual)
            x2s = pp.tile([C, NPIX], f32, tag="x2s")      # x + attn residual
            xin = pp.tile([C, NPIX], bf16, tag="xin")     # normalized relu(LN1(x))
            rec0s = pp.tile([C, NPIX], bf16, tag="rec0s")  # iter-0 reciprocal
            # fp8 padded h image, TWO copies: copy1's row r holds copy0's
            # row r+1 so the (dy=0, dy=1) DoubleRow pair is a clean
            # stride-PADPIX8 AP dimension
            hpad = pp.tile([C, 2 * PADPIX8], fp8, tag="hpad")
            rpad = pp.tile([C, PADPIX], bf16, tag="rpad")
            wr = pp.tile([C, 9 * C], fp8, tag="wr")
            wc = pp.tile([C, 9 * C], bf16, tag="wc")
            w1s = pp.tile([C, HID], bf16, tag="w1s")
            w2s = [pp.tile([128, C], bf16, tag=f"w2s{k}", name=f"w2s{k}")
                   for k in range(3)]
            b1s = pp.tile([128, 3], f32, tag="b1s")
            b2s = pp.tile([C, 1], f32, tag="b2s")
            ln1w = pp.tile([C, 1], f32, tag="ln1w")
            ln1b = pp.tile([C, 1], f32, tag="ln1b")
            ones_sq = pp.tile([C, C], bf16, tag="ones_sq")     # sum+bcast lhsT
            # selector stationaries: sel1 block c = (96,7) with column c ones
            # (accumulate chunk c's channel-sum into psum partition c); selrow
            # block c = (7,96) with row c ones (broadcast stats row c to all
            # 96 partitions).
            sel1 = pp.tile([C, NCHUNK * NCHUNK], bf16, tag="sel1")
            selrow = pp.tile([NCHUNK, NCHUNK * C], bf16, tag="selrow")
            eps6 = pp.tile([16, 1], f32, tag="eps6")           # 1e-6 (LN1)
            eps5 = pp.tile([16, 1], f32, tag="eps5")           # 1e-5 (LN2)
            warm = pp.tile([128, 576], bf16, tag="warm")       # HAM warm-up junk
            # batched LayerNorm statistics rows (chunk c on partition c)
            ln_u = pp.tile([NCHUNK, CW], bf16, tag="ln_u")
            ln_isd = pp.tile([NCHUNK, CW], bf16, tag="ln_isd")
            ln_u2 = pp.tile([NCHUNK, CW], f32, tag="ln_u2")
            ln_var = pp.tile([NCHUNK, CW], f32, tag="ln_var")
            m_u = pp.tile([NCHUNK, CW], bf16, tag="m_u")
            m_isd = pp.tile([NCHUNK, CW], bf16, tag="m_isd")
            m_u2 = pp.tile([NCHUNK, CW], f32, tag="m_u2")
            m_var = pp.tile([NCHUNK, CW], f32, tag="m_var")

            # ---- HAM warm-up: ~5us of dense junk matmuls so the PE clock
            # gate opens before the LayerNorm prologue (otherwise the first
            # ~90us run at 1.2 GHz). Runs while the input DMAs land. ----
            nc.vector.memset(warm[:], 0.0)
            wps = ps_fill.tile([128, CW], f32, tag="fill", name="warmps")
            for _ in range(24):
                nc.tensor.matmul(wps[:], warm[:, 0:128], warm[:, 128:576])

            def fillers(n):
                # keep-warm junk matmuls into the (consumer-less) warm-up
                # psum tile: the LN/MLP phases leave the PE below the HAM
                # busy threshold, and once the clock gate closes every real
                # matmul runs at half rate.
                for _ in range(n):
                    nc.tensor.matmul(wps[:], warm[:, 0:128], warm[:, 128:576])

            # ---- load inputs. Sync queue: LN params + x chunks + conv
            # weights, ordered by first use. gpsimd queue takes the rest. ----
            nc.sync.dma_start(ln1w[:], ln1w_d[:])
            nc.sync.dma_start(ln1b[:], ln1b_d[:])
            nc.sync.dma_start(sel1[:], sel1_d[:])
            nc.sync.dma_start(selrow[:], selrow_d[:])
            nc.sync.dma_start(xs[:, 0:CW], x_d[:, 0:CW])
            nc.sync.dma_start(wc[:], wc_d[:])
            for c in range(1, NCHUNK):
                nc.sync.dma_start(xs[:, c * CW:(c + 1) * CW],
                                  x_d[:, c * CW:(c + 1) * CW])
            nc.sync.dma_start(wr[:], wr_d[:])
            nc.gpsimd.dma_start(rec0s[:], rec0_d[:])
            nc.gpsimd.dma_start(w1s[:], w1_d[:])
            for k in range(3):
                nc.gpsimd.dma_start(w2s[k][:], w2_d[k * 128:(k + 1) * 128, :])
            nc.gpsimd.dma_start(b1s[:],
                                b1_d[:].rearrange("(k p) one -> p (k one)", p=128))
            nc.gpsimd.dma_start(b2s[:], b2_d[:])

            nc.vector.memset(ones_sq[:], 1.0 / QS)
            nc.vector.memset(eps6[:], 1e-6)
            nc.vector.memset(eps5[:], 1e-5)

            def pad3(t):
                return t[:].rearrange("p (h w) -> p h w", h=HP)

            def interior(t, y0, nrows):
                return pad3(t)[0:C, 1 + y0:1 + y0 + nrows, 1:1 + W]

            # 3D views of the two fp8 h-image copies and the 4D pair view
            hc0 = hpad[:, 0:PADPIX8].rearrange("p (h w) -> p h w", h=HP)
            hc1 = hpad[:, PADPIX8:2 * PADPIX8].rearrange("p (h w) -> p h w",
                                                         h=HP)
            hp4 = hpad[:].rearrange("p (two h w) -> p two h w", two=2, h=HP)

            def hint0(y0, nrows):
                return hc0[0:C, 1 + y0:1 + y0 + nrows, 1:1 + W]

            def hint1(y0, nrows):
                return hc1[0:C, y0:y0 + nrows, 1:1 + W]

            # border-only init: the interiors are covered by the SH/C fill
            # (hpad) and by the ratio multiplies (rpad), so only the zero
            # borders need explicit memsets.
            nc.vector.memset(pad3(rpad)[0:C, 0:1, :], 0.0)
            nc.vector.memset(pad3(rpad)[0:C, HP - 1:HP, :], 0.0)
            nc.vector.memset(pad3(rpad)[0:C, 1:HP - 1, 0:1], 0.0)
            nc.vector.memset(pad3(rpad)[0:C, 1:HP - 1, HP - 1:HP], 0.0)
            nc.vector.memset(hc0[0:C, 0:1, 0:HP], 0.0)            # top row
            nc.vector.memset(hc0[0:C, HP - 1:HP, 0:HP], 0.0)      # bottom row
            nc.vector.memset(hc0[0:C, 1:HP - 1, 0:1], 0.0)        # left col
            nc.vector.memset(hc0[0:C, 1:HP - 1, HP - 1:HP], 0.0)  # right col
            nc.vector.memset(hc1[0:C, HP - 2:HP, 0:HP], 0.0)      # rows 56,57
            nc.vector.memset(hc1[0:C, 0:HP - 2, 0:1], 0.0)        # left col
            nc.vector.memset(hc1[0:C, 0:HP - 2, HP - 1:HP], 0.0)  # right col
            nc.vector.memset(hint0(0, H), SH / C)
            nc.vector.memset(hint1(0, H), SH / C)

            def sumbcast(src_ap, name="sb"):
                """ones 96x96 matmul: out[o,p] = sum_c src[c,p] for all o."""
                b = ps_big.tile([C, CW], f32, tag="bcast", name=name)
                nc.tensor.matmul(b[:], ones_sq[:], src_ap)
                return b

            def bcast_row(rows_t, c, name="bc"):
                """broadcast row c of a (7, CW) stats tile to all 96
                partitions: selector (7,96) lhsT with row c all-ones."""
                b = ps_big.tile([C, CW], f32, tag="bcast", name=name)
                nc.tensor.matmul(b[:], selrow[:, C * c:C * (c + 1)],
                                 rows_t[0:NCHUNK, :])
                return b

            # ---- LN stats, phase 1: chunk c's channel-sums of x and x^2
            # accumulate into psum partition c of two (7, CW) tiles (selector
            # stationary: column c ones). ----
            def stats_mms(pstatA, pstatB, c, xc_f32, xbc):
                sq = wp.tile([C, CW], bf16, tag="ln_sq")
                nc.scalar.square(sq[:], xc_f32)
                sel = sel1[:, NCHUNK * c:NCHUNK * (c + 1)]
                nc.tensor.matmul(pstatA[0:NCHUNK, :], sel, xbc,
                                 start=(c == 0), stop=(c == NCHUNK - 1))
                nc.tensor.matmul(pstatB[0:NCHUNK, :], sel, sq[:],
                                 start=(c == 0), stop=(c == NCHUNK - 1))

            # ---- LN stats, phase 2 (batched): mean/istd for all chunks ----
            def stats_rows(pstatA, pstatB, u_t, u2_t, var_t, isd_t, eps_t):
                nc.scalar.activation(u2_t[:], pstatA[0:NCHUNK, :], AF.Square,
                                     scale=1.0 / C)
                nc.vector.scalar_tensor_tensor(
                    var_t[:], pstatB[0:NCHUNK, :], 1.0 / C, u2_t[:],
                    OP.mult, OP.subtract)
                sd_t = sp.tile([NCHUNK, CW], f32, tag="sd")
                nc.scalar.activation(sd_t[:], var_t[:], AF.Sqrt,
                                     bias=eps_t[0:NCHUNK, 0:1])
                isdf_t = sp.tile([NCHUNK, CW], f32, tag="isdf")
                nc.vector.reciprocal_approx_fast(out=isdf_t[:], in_=sd_t[:])
                nc.vector.tensor_copy(isd_t[:], isdf_t[:])
                nc.scalar.activation(u_t[:], pstatA[0:NCHUNK, :], AF.Identity,
                                     scale=1.0 / C)

            # ---- LN1 per-chunk normalize + relu + channel-normalize ----
            def ln1_finish(c):
                sl = slice(c * CW, (c + 1) * CW)
                xc = xs[:, sl]
                ub = bcast_row(ln_u, c, name="ubc")
                ib = bcast_row(ln_isd, c, name="ibc")
                xm = wp.tile([C, CW], f32, tag="ln_xm")
                nc.vector.tensor_tensor(xm[:], xc, ub[:], OP.subtract)
                xn = wp.tile([C, CW], f32, tag="ln_xn")
                nc.vector.tensor_tensor(xn[:], xm[:], ib[:], OP.mult)
                rl = wp.tile([C, CW], bf16, tag="ln_rl")
                nc.scalar.activation(rl[:], xn[:], AF.Relu,
                                     bias=ln1b[:, 0:1], scale=ln1w[:, 0:1])
                sb = sumbcast(rl[:], name="lnsb")
                rb = wp.tile([C, CW], f32, tag="ln_rb")
                nc.vector.reciprocal_approx_fast(out=rb[:], in_=sb[:])
                nc.vector.tensor_tensor(xin[:, sl], rl[:], rb[:], OP.mult)

            # ---- NNMF phase builders ----
            def phase_a(c):
                # recon = convT(h): three fp8 DoubleRow matmuls carry the
                # (dy=0, dy=1) pairs, three normal fp8 matmuls the dy=2 row
                y0 = c * CH
                ps = ps_conv.tile([C, CW], f32, tag="conv", name="psA")
                for dx in range(3):
                    lhsT = wr[:, dx * 2 * C:(dx + 1) * 2 * C].rearrange(
                        "p (two m) -> p two m", two=2)
                    rhs = hp4[0:C, 0:2, y0:y0 + CH, dx:dx + W]
                    nc.tensor.matmul(ps[:], lhsT, rhs, start=(dx == 0),
                                     stop=False, perf_mode=DR)
                for dx in range(3):
                    view = hc0[0:C, y0 + 2:y0 + 2 + CH, dx:dx + W]
                    nc.tensor.matmul(ps[:], wr[:, 6 * C + dx * C:
                                                6 * C + (dx + 1) * C], view,
                                     start=False, stop=(dx == 2))
                # recon >= ~1e-5 everywhere (strictly positive weights and h),
                # so the reference's +1e-12 guard is numerically irrelevant
                # and the reciprocal reads PSUM directly.
                rec = wp.tile([C, CW], f32, tag="rec", bufs=6)
                nc.vector.reciprocal_approx_fast(out=rec[:], in_=ps[:])
                ratio_mult(c, rec)

            def ratio_mult(c, rec):
                # ratio = xin * (1/recon), on gpsimd (latency hidden by skew)
                y0 = c * CH
                nc.gpsimd.tensor_tensor(
                    interior(rpad, y0, CH),
                    xin[:, c * CW:(c + 1) * CW], rec[:], OP.mult)

            def phase_b1(c):
                # conv(ratio) and ht = h * conv
                y0 = c * CH
                ps = ps_conv.tile([C, CW], f32, tag="conv", name="psB")
                for k in range(9):
                    dy, dx = k // 3, k % 3
                    view = pad3(rpad)[0:C, y0 + dy:y0 + dy + CH, dx:dx + W]
                    nc.tensor.matmul(ps[:], wc[:, k * C:(k + 1) * C], view,
                                     start=(k == 0), stop=(k == 8))
                ht = wp.tile([C, CW], bf16, tag="ht", bufs=6)
                nc.vector.tensor_tensor(ht[:], hint0(y0, CH), ps[:], OP.mult)
                return ht

            def phase_b2(c, ht):
                # channel sum broadcast to every partition in one matmul
                # (ones 96x96 stationary), then 1/S on the DVE
                sb = sumbcast(ht[:], name="nsb")
                rb = wp.tile([C, CW], f32, tag="nrb", bufs=6)
                nc.vector.reciprocal_approx_fast(out=rb[:], in_=sb[:])
                return rb

            def phase_b3(c, ht, rb):
                # hpad_new = SH * h_new: ht carries SH*h*conv, rb carries
                # QS/(SH*S), so the stt scalar is SH/QS... both fp8 copies.
                y0 = c * CH
                nc.vector.scalar_tensor_tensor(
                    hint0(y0, CH), ht[:], SH / QS, rb[:], OP.mult, OP.mult)
                nc.vector.scalar_tensor_tensor(
                    hint1(y0, CH), ht[:], SH / QS, rb[:], OP.mult, OP.mult)

            # ---- LN2 + MLP + residual ----
            def mlp_p1(pstatA, pstatB, c):
                # x2 = x + h (hpad stores SH*h in fp8; gpsimd cannot read
                # fp8, so the residual add runs on the DVE in the tail)
                sl = slice(c * CW, (c + 1) * CW)
                nc.vector.scalar_tensor_tensor(
                    x2s[:, sl], hint0(c * CH, CH), 1.0 / SH, xs[:, sl],
                    OP.mult, OP.add)
                xc = x2s[:, sl]
                x2b = wp.tile([C, CW], bf16, tag="x2b")
                nc.vector.tensor_copy(x2b[:], xc)
                stats_mms(pstatA, pstatB, c, xc, x2b[:])

            def mlp_p2(c):
                sl = slice(c * CW, (c + 1) * CW)
                xc = x2s[:, sl]
                ub = bcast_row(m_u, c, name="ubc")
                ib = bcast_row(m_isd, c, name="ibc")
                xm = wp.tile([C, CW], f32, tag="ln_xm")
                nc.vector.tensor_tensor(xm[:], xc, ub[:], OP.subtract)
                # LN2's affine is folded into w1/b1 on the host, so the
                # normalized value feeds the matmul directly (as bf16).
                xn = wp.tile([C, CW], bf16, tag="ln_xw", bufs=8)
                nc.vector.tensor_tensor(xn[:], xm[:], ib[:], OP.mult)
                return xn

            def mlp_p3(c, xn):
                ys = []
                for j in range(3):
                    p1 = ps_big.tile([128, CW], f32, tag="bcast", name="p1")
                    nc.tensor.matmul(p1[:], w1s[:, j * 128:(j + 1) * 128], xn[:])
                    y1 = wp.tile([128, CW], bf16, tag=f"mlp_y{j}", name=f"mlp_y{j}")
                    if gelu_mode == "hw":
                        nc.scalar.activation(y1[:], p1[:], AF.Gelu,
                                             bias=b1s[:, j:j + 1])
                    else:
                        # CoreSim fallback: sigmoid-GELU (Gelu not implemented
                        # in the simulator). Mirror must match.
                        pre = wp.tile([128, CW], f32, tag=f"mlp_p{j}",
                                      name=f"mlp_p{j}")
                        nc.scalar.activation(pre[:], p1[:], AF.Identity,
                                             bias=b1s[:, j:j + 1])
                        sg = wp.tile([128, CW], f32, tag=f"mlp_s{j}",
                                     name=f"mlp_s{j}")
                        nc.scalar.activation(sg[:], pre[:], AF.Sigmoid,
                                             scale=1.702)
                        nc.vector.tensor_tensor(y1[:], pre[:], sg[:], OP.mult)
                    ys.append(y1)
                return ys

            def mlp_p4(c, ys):
                sl = slice(c * CW, (c + 1) * CW)
                p2 = ps_conv.tile([C, CW], f32, tag="conv")
                for k in range(3):
                    nc.tensor.matmul(p2[:], w2s[k][:], ys[k][:],
                                     start=(k == 0), stop=(k == 2))
                oc = wp.tile([C, CW], f32, tag="oc")
                nc.vector.scalar_tensor_tensor(
                    oc[:], p2[:], b2s[:, 0:1], x2s[:, sl], OP.add, OP.add)
                if c % 2 == 0:
                    nc.sync.dma_start(out_d[:, sl], oc[:])
                else:
                    nc.gpsimd.dma_start(out_d[:, sl], oc[:])

            # ---- LN1 stats phase (prologue, batched rows) ----
            pstat1A = ps_stat.tile([NCHUNK, CW], f32, tag="statA",
                                   name="pstat1A")
            pstat1B = ps_stat.tile([NCHUNK, CW], f32, tag="statB",
                                   name="pstat1B")
            for c in range(NCHUNK):
                sl = slice(c * CW, (c + 1) * CW)
                xbc = wp.tile([C, CW], bf16, tag="x2b")
                nc.vector.tensor_copy(xbc[:], xs[:, sl])
                stats_mms(pstat1A, pstat1B, c, xs[:, sl], xbc[:])
                fillers(3)
            stats_rows(pstat1A, pstat1B, ln_u, ln_u2, ln_var, ln_isd, eps6)
            fillers(6)

            # ---- ONE global software pipeline: LN1-finish chunks play the
            # A-stage role for iteration 0 (its back-projection reciprocal is
            # the precomputed rec0), then every NNMF chunk-slot, then the MLP
            # stages ride the tail. ----
            total = nit * NCHUNK
            hts = {}
            rbs = {}
            xns = {}
            yss = {}
            pstat2A = pstat2B = None
            for s in range(0, total + NCHUNK + 11):
                if s < min(NCHUNK, total):
                    ln1_finish(s)
                    ratio_mult(s, rec0s[:, s * CW:(s + 1) * CW])
                    fillers(5)
                elif s < total:
                    phase_a(s % NCHUNK)
                if total <= s:
                    # MLP tail slots are matmul-sparse; hold the clock open
                    fillers(4)
                c1 = s - 2
                if 0 <= c1 < total:
                    hts[c1] = phase_b1(c1 % NCHUNK)
                c2 = s - 3
                if 0 <= c2 < total:
                    rbs[c2] = phase_b2(c2 % NCHUNK, hts[c2])
                c3 = s - 4
                if 0 <= c3 < total:
                    phase_b3(c3 % NCHUNK, hts.pop(c3), rbs.pop(c3))
                # MLP stats (Square on ACT, sums on PE) trail the last
                # iteration's b3 slots; the batched row stage runs once after
                # all 7 chunks' sums are in.
                m1 = s - (total - 2)
                if 0 <= m1 < NCHUNK:
                    if pstat2A is None:
                        pstat2A = ps_stat.tile([NCHUNK, CW], f32,
                                               tag="statA", name="pstat2A")
                        pstat2B = ps_stat.tile([NCHUNK, CW], f32,
                                               tag="statB", name="pstat2B")
                    mlp_p1(pstat2A, pstat2B, m1)
                if m1 == NCHUNK:
                    stats_rows(pstat2A, pstat2B, m_u, m_u2, m_var, m_isd, eps5)
                m2 = s - (total + NCHUNK - 2)
                if 0 <= m2 < NCHUNK:
                    xns[m2] = mlp_p2(m2)
                m3 = s - (total + NCHUNK - 1)
                if 0 <= m3 < NCHUNK:
                    yss[m3] = mlp_p3(m3, xns.pop(m3))
                m4 = s - (total + NCHUNK)
                if 0 <= m4 < NCHUNK:
                    mlp_p4(m4, yss.pop(m4))

    return nc


def _prepare_maps(x, ln1_w, ln1_b, w_nnmf, ln2_w, ln2_b, w1, b1, w2, b2):
    import ml_dtypes
    bf16 = ml_dtypes.bfloat16
    WcD, Wr8, rec0 = _build_conv_mats(w_nnmf)
    f = lambda a: np.ascontiguousarray(np.asarray(a, np.float32))
    fb = lambda a: np.ascontiguousarray(np.asarray(a, np.float32).astype(bf16))
    f8 = lambda a: np.ascontiguousarray(
        np.asarray(a, np.float32).astype(ml_dtypes.float8_e4m3))
    # LN2's per-channel affine folded into the first MLP matmul:
    # (xn*w + b) @ w1 + b1 == xn @ (diag(w) @ w1) + (b1 + b @ w1)
    w1_64 = np.asarray(w1, np.float64)
    w1f = w1_64 * np.asarray(ln2_w, np.float64)[:, None]
    b1f = np.asarray(b1, np.float64) + np.asarray(ln2_b, np.float64) @ w1_64
    # selector stationaries (see _build_bass)
    sel1 = np.zeros((C, NCHUNK * NCHUNK), np.float32)
    selrow = np.zeros((NCHUNK, NCHUNK * C), np.float32)
    for c in range(NCHUNK):
        sel1[:, NCHUNK * c + c] = 1.0
        selrow[c, C * c:C * (c + 1)] = 1.0
    shared = {
        "sel1": fb(sel1),
        "selrow": fb(selrow),
        "rec0": fb(rec0),
        "wrecon": f8(Wr8),
        "wconv": fb(WcD),
        "w1T": fb(w1f),
        "b1": f(b1f).reshape(HID, 1),
        "w2T": fb(w2),
        "b2": f(b2).reshape(C, 1),
        "ln1w": f(ln1_w).reshape(C, 1),
        "ln1b": f(ln1_b).reshape(C, 1),
    }
    xs = np.asarray(x)
    return [dict(shared, x=f(xs[i]).reshape(C, NPIX))
            for i in range(xs.shape[0])]


def kernel(x, ln1_w, ln1_b, w_nnmf, ln2_w, ln2_b, w1, b1, w2, b2):
    global _CACHED_NC, LAST_RESULT
    from concourse.bass_utils import run_bass_kernel_spmd

    if _CACHED_NC is None:
        nc = _build_bass()
        nc.finalize()
        _CACHED_NC = nc
    nc = _CACHED_NC
    in_maps = _prepare_maps(x, ln1_w, ln1_b, w_nnmf, ln2_w, ln2_b, w1, b1, w2, b2)
    res = run_bass_kernel_spmd(nc, in_maps, core_ids=list(range(8)), trace=TRACE)
    LAST_RESULT = res
    out = np.stack([res.results[i]["out"].reshape(C, H, W) for i in range(8)])
    return out.astype(np.float32)


# revision 43
# speedup vs baseline: 1.3643x; 1.0036x over previous
"""Trainium2 Bass kernel for nn_Block_17978733101066.

ConvNeXt-style block: channels-first LayerNorm -> NNMF conv (25 multiplicative
updates with grouped 3x3 convs) residual branch, then channels-last LayerNorm +
MLP residual branch.  Input x: (8, 96, 56, 56) f32.

Strategy: pure data parallel - one sample per NeuronCore (8 cores).  Per-core
layout keeps channels on SBUF partitions (C=96) and flattened spatial
positions on the free axis, chunked 448 wide.  Each grouped 3x3 conv is 9
PSUM-accumulated bf16 matmuls with per-offset block-diagonal (96x96) weight
matrices (host-built) against shifted views of a zero-padded (58x58) bf16
SBUF image.

Key scheduling/engine choices vs the naive version:
 - a ~5us dense warm-up matmul burst at kernel start keeps the PE HAM clock
   gate at 8/8 through the (otherwise sparse) LayerNorm prologue;
 - the per-chunk LayerNorm statistics rows are accumulated into adjacent
   PSUM partitions so the scalar follow-ups (mean/var/rsqrt) run once over a
   (7,448) tile instead of 7x over (1,448) rows;
 - channel-sum + broadcast is ONE matmul with an all-ones 96x96 stationary
   operand (out[o,p] = sum_c in[c,p] for every o), halving the PE overhead
   of the NNMF renormalisation;
 - all reciprocals run on the otherwise idle ACT engine (AF.Reciprocal with
   the eps as activation bias), keeping the DVE off the critical path;
 - x is DMA'd chunk-wise so the first LayerNorm chunk starts ~1.5us in.

Iteration 0's back-projection depends only on the constant h0 and ships as a
precomputed reciprocal.  All residual-path arithmetic stays f32.
"""

import numpy as np

C = 96
H = W = 56
NPIX = H * W          # 3136
HP = H + 2            # 58
PADPIX = HP * HP      # 3364
WP8 = 64              # fp8 padded-image row stride (16B-aligned pairs)
PADPIX8 = HP * WP8    # 3712
G, CG = 4, 24
NIT = 25
EPS = 1e-12
CH = 8                # image rows per chunk
NCHUNK = H // CH      # 7
CW = CH * W           # 448 positions per chunk
HID = 384
# fp8 scale plan: hpad stores 64*h, recon weights store 512*wr, the ones
# sum+bcast matrix stores 1/8192 (so xin carries 8192x and the normalizer
# reciprocals absorb the rest); conv weights store 4*wc so psB is the true
# conv(ratio).
SH, SW, QS = 64.0, 512.0, 8192.0

TRACE = False         # set True (e.g. from test.py) to collect NTFF exec time
LAST_RESULT = None    # BassKernelResults of the most recent run

_CACHED_NC = None


def _build_conv_mats(w_nnmf):
    """Per-offset lhsT matrices for both convs, packed (96, 9*96) f32."""
    w = np.abs(np.asarray(w_nnmf, np.float64))
    w = w / (w.sum(axis=(1, 2, 3), keepdims=True) + EPS)  # (96, 24, 3, 3)
    Wc = np.zeros((9, C, C), np.float64)  # [k, i, o] = w[o, i_loc, dy, dx]
    Wr = np.zeros((9, C, C), np.float64)  # [k, o, i] = w[o, i_loc, 2-dy, 2-dx]
    for dy in range(3):
        for dx in range(3):
            k = dy * 3 + dx
            blkc = w[:, :, dy, dx]          # (96 out, 24 in_local)
            blkr = w[:, :, 2 - dy, 2 - dx]  # (96 out, 24 in_local)
            for g in range(G):
                rows = slice(g * CG, (g + 1) * CG)
                Wc[k, rows, rows] = blkc[rows, :].T
                Wr[k, rows, rows] = blkr[rows, :]
    # conv(ratio) stationaries: x4 so psB is conv(ratio_true) given the
    # stored ratio is ratio/4
    WcD = np.ascontiguousarray(
        4.0 * Wc.transpose(1, 0, 2).reshape(C, 9 * C), np.float32)
    # recon stationaries in fp8 DoubleRow layout: three (dy0|dy1) pair
    # blocks (one per dx), then the three dy=2 singles; values x512 to
    # clear the fp8e4 denormal range
    Wr8 = np.zeros((C, 9 * C), np.float64)
    for dx in range(3):
        Wr8[:, dx * 2 * C:dx * 2 * C + C] = Wr[0 * 3 + dx]
        Wr8[:, dx * 2 * C + C:(dx + 1) * 2 * C] = Wr[1 * 3 + dx]
        Wr8[:, 6 * C + dx * C:6 * C + (dx + 1) * C] = Wr[2 * 3 + dx]
    Wr8 = np.ascontiguousarray(SW * Wr8, np.float32)
    # iteration-0 back-projection is data independent (h0 is the constant
    # 1/C fill): ship 1/(SH*SW*(convT(h0) + eps)) as a precomputed input
    hpad0 = np.zeros((C, HP, HP))
    hpad0[:, 1:1 + H, 1:1 + W] = 1.0 / C
    recon0 = np.zeros((C, H * W))
    for dy in range(3):
        for dx in range(3):
            k = dy * 3 + dx
            view = hpad0[:, dy:dy + H, dx:dx + W].reshape(C, H * W)
            recon0 += Wr[k].T @ view
    rec0 = (1.0 / (SH * SW * (recon0 + EPS))).astype(np.float32)
    return WcD, Wr8, np.ascontiguousarray(rec0)


def _build_bass(nit=NIT, gelu_mode="hw"):
    import concourse.bass as bass
    import concourse.bacc as bacc
    import concourse.mybir as mybir
    from concourse.tile import TileContext

    f32 = mybir.dt.float32
    bf16 = mybir.dt.bfloat16
    fp8 = mybir.dt.float8e4
    AF = mybir.ActivationFunctionType
    OP = mybir.AluOpType
    DR = mybir.MatmulPerfMode.DoubleRow

    nc = bacc.Bacc(None, target_bir_lowering=False)

    x_d = nc.declare_dram_parameter("x", [C, NPIX], f32, isOutput=False)
    rec0_d = nc.declare_dram_parameter("rec0", [C, NPIX], bf16, isOutput=False)
    wr_d = nc.declare_dram_parameter("wrecon", [C, 9 * C], fp8, isOutput=False)
    wc_d = nc.declare_dram_parameter("wconv", [C, 9 * C], bf16, isOutput=False)
    w1_d = nc.declare_dram_parameter("w1T", [C, HID], bf16, isOutput=False)
    b1_d = nc.declare_dram_parameter("b1", [HID, 1], f32, isOutput=False)
    w2_d = nc.declare_dram_parameter("w2T", [HID, C], bf16, isOutput=False)
    b2_d = nc.declare_dram_parameter("b2", [C, 1], f32, isOutput=False)
    ln1w_d = nc.declare_dram_parameter("ln1w", [C, 1], f32, isOutput=False)
    ln1b_d = nc.declare_dram_parameter("ln1b", [C, 1], f32, isOutput=False)
    sel1_d = nc.declare_dram_parameter("sel1", [C, NCHUNK * NCHUNK], bf16,
                                       isOutput=False)
    selrow_d = nc.declare_dram_parameter("selrow", [NCHUNK, NCHUNK * C], bf16,
                                         isOutput=False)
    out_d = nc.declare_dram_parameter("out", [C, NPIX], f32, isOutput=True)

    with TileContext(nc) as tc:
        with (
            tc.tile_pool(name="persist", bufs=1) as pp,
            tc.tile_pool(name="work", bufs=4) as wp,
            tc.tile_pool(name="small", bufs=3) as sp,
            tc.tile_pool(name="psconv", bufs=3, space="PSUM") as ps_conv,
            tc.tile_pool(name="psbig", bufs=2, space="PSUM") as ps_big,
            tc.tile_pool(name="psstat", bufs=1, space="PSUM") as ps_stat,
            tc.tile_pool(name="psfill", bufs=1, space="PSUM") as ps_fill,
        ):
            # ---- persistent tiles ----
            xs = pp.tile([C, NPIX], f32, tag="xs")        # original x (residual)
            x2s = pp.tile([C, NPIX], f32, tag="x2s")      # x + attn residual
            hfin = pp.tile([C, NPIX], f32, tag="hfin")    # final-iteration h
            xin = pp.tile([C, NPIX], bf16, tag="xin")     # normalized relu(LN1(x))
            rec0s = pp.tile([C, NPIX], bf16, tag="rec0s")  # iter-0 reciprocal
            # fp8 padded h image, TWO copies: copy1's row r holds copy0's
            # row r+1 so the (dy=0, dy=1) DoubleRow pair is a clean
            # stride-PADPIX8 AP dimension
            hpad = pp.tile([C, 2 * PADPIX8], fp8, tag="hpad")
            rpad = pp.tile([C, PADPIX], bf16, tag="rpad")
            wr = pp.tile([C, 9 * C], fp8, tag="wr")
            wc = pp.tile([C, 9 * C], bf16, tag="wc")
            w1s = pp.tile([C, HID], bf16, tag="w1s")
            w2s = [pp.tile([128, C], bf16, tag=f"w2s{k}", name=f"w2s{k}")
                   for k in range(3)]
            b1s = pp.tile([128, 3], f32, tag="b1s")
            b2s = pp.tile([C, 1], f32, tag="b2s")
            ln1w = pp.tile([C, 1], f32, tag="ln1w")
            ln1b = pp.tile([C, 1], f32, tag="ln1b")
            ones_sq = pp.tile([C, C], bf16, tag="ones_sq")     # sum+bcast lhsT
            # selector stationaries: sel1 block c = (96,7) with column c ones
            # (accumulate chunk c's channel-sum into psum partition c); selrow
            # block c = (7,96) with row c ones (broadcast stats row c to all
            # 96 partitions).
            sel1 = pp.tile([C, NCHUNK * NCHUNK], bf16, tag="sel1")
            selrow = pp.tile([NCHUNK, NCHUNK * C], bf16, tag="selrow")
            eps6 = pp.tile([16, 1], f32, tag="eps6")           # 1e-6 (LN1)
            eps5 = pp.tile([16, 1], f32, tag="eps5")           # 1e-5 (LN2)
            warm = pp.tile([128, 576], bf16, tag="warm")       # HAM warm-up junk
            # batched LayerNorm statistics rows (chunk c on partition c)
            ln_u = pp.tile([NCHUNK, CW], bf16, tag="ln_u")
            ln_isd = pp.tile([NCHUNK, CW], bf16, tag="ln_isd")
            ln_u2 = pp.tile([NCHUNK, CW], f32, tag="ln_u2")
            ln_var = pp.tile([NCHUNK, CW], f32, tag="ln_var")
            m_u = pp.tile([NCHUNK, CW], bf16, tag="m_u")
            m_isd = pp.tile([NCHUNK, CW], bf16, tag="m_isd")
            m_u2 = pp.tile([NCHUNK, CW], f32, tag="m_u2")
            m_var = pp.tile([NCHUNK, CW], f32, tag="m_var")

            # ---- HAM warm-up: ~5us of dense junk matmuls so the PE clock
            # gate opens before the LayerNorm prologue (otherwise the first
            # ~90us run at 1.2 GHz). Runs while the input DMAs land. ----
            nc.vector.memset(warm[:], 0.0)
            wps = ps_fill.tile([128, CW], f32, tag="fill", name="warmps")
            for _ in range(24):
                nc.tensor.matmul(wps[:], warm[:, 0:128], warm[:, 128:576])

            def fillers(n):
                # keep-warm junk matmuls into the (consumer-less) warm-up
                # psum tile: the LN/MLP phases leave the PE below the HAM
                # busy threshold, and once the clock gate closes every real
                # matmul runs at half rate.
                for _ in range(n):
                    nc.tensor.matmul(wps[:], warm[:, 0:128], warm[:, 128:576])

            # ---- load inputs. Sync queue: LN params + x chunks + conv
            # weights, ordered by first use. gpsimd queue takes the rest. ----
            nc.sync.dma_start(ln1w[:], ln1w_d[:])
            nc.sync.dma_start(ln1b[:], ln1b_d[:])
            nc.sync.dma_start(sel1[:], sel1_d[:])
            nc.sync.dma_start(selrow[:], selrow_d[:])
            nc.sync.dma_start(xs[:, 0:CW], x_d[:, 0:CW])
            nc.sync.dma_start(wc[:], wc_d[:])
            for c in range(1, NCHUNK):
                nc.sync.dma_start(xs[:, c * CW:(c + 1) * CW],
                                  x_d[:, c * CW:(c + 1) * CW])
            nc.sync.dma_start(wr[:], wr_d[:])
            nc.gpsimd.dma_start(rec0s[:], rec0_d[:])
            nc.gpsimd.dma_start(w1s[:], w1_d[:])
            for k in range(3):
                nc.gpsimd.dma_start(w2s[k][:], w2_d[k * 128:(k + 1) * 128, :])
            nc.gpsimd.dma_start(b1s[:],
                                b1_d[:].rearrange("(k p) one -> p (k one)", p=128))
            nc.gpsimd.dma_start(b2s[:], b2_d[:])

            nc.vector.memset(ones_sq[:], 1.0 / QS)
            nc.vector.memset(eps6[:], 1e-6)
            nc.vector.memset(eps5[:], 1e-5)

            def pad3(t):
                return t[:].rearrange("p (h w) -> p h w", h=HP)

            def interior(t, y0, nrows):
                return pad3(t)[0:C, 1 + y0:1 + y0 + nrows, 1:1 + W]

            # 3D views of the two fp8 h-image copies and the 4D pair view
            hc0 = hpad[:, 0:PADPIX8].rearrange("p (h w) -> p h w", h=HP)
            hc1 = hpad[:, PADPIX8:2 * PADPIX8].rearrange("p (h w) -> p h w",
                                                         h=HP)
            hp4 = hpad[:].rearrange("p (two h w) -> p two h w", two=2, h=HP)

            def hint0(y0, nrows):
                return hc0[0:C, 1 + y0:1 + y0 + nrows, 1:1 + W]

            def hint1(y0, nrows):
                return hc1[0:C, y0:y0 + nrows, 1:1 + W]

            # border-only init: the interiors are covered by the SH/C fill
            # (hpad) and by the ratio multiplies (rpad), so only the zero
            # borders need explicit memsets.
            nc.vector.memset(pad3(rpad)[0:C, 0:1, :], 0.0)
            nc.vector.memset(pad3(rpad)[0:C, HP - 1:HP, :], 0.0)
            nc.vector.memset(pad3(rpad)[0:C, 1:HP - 1, 0:1], 0.0)
            nc.vector.memset(pad3(rpad)[0:C, 1:HP - 1, HP - 1:HP], 0.0)
            nc.vector.memset(hc0[0:C, 0:1, 0:HP], 0.0)            # top row
            nc.vector.memset(hc0[0:C, HP - 1:HP, 0:HP], 0.0)      # bottom row
            nc.vector.memset(hc0[0:C, 1:HP - 1, 0:1], 0.0)        # left col
            nc.vector.memset(hc0[0:C, 1:HP - 1, HP - 1:HP], 0.0)  # right col
            nc.vector.memset(hc1[0:C, HP - 2:HP, 0:HP], 0.0)      # rows 56,57
            nc.vector.memset(hc1[0:C, 0:HP - 2, 0:1], 0.0)        # left col
            nc.vector.memset(hc1[0:C, 0:HP - 2, HP - 1:HP], 0.0)  # right col
            nc.vector.memset(hint0(0, H), SH / C)
            nc.vector.memset(hint1(0, H), SH / C)

            def sumbcast(src_ap, name="sb"):
                """ones 96x96 matmul: out[o,p] = sum_c src[c,p] for all o."""
                b = ps_big.tile([C, CW], f32, tag="bcast", name=name)
                nc.tensor.matmul(b[:], ones_sq[:], src_ap)
                return b

            def bcast_row(rows_t, c, name="bc"):
                """broadcast row c of a (7, CW) stats tile to all 96
                partitions: selector (7,96) lhsT with row c all-ones."""
                b = ps_big.tile([C, CW], f32, tag="bcast", name=name)
                nc.tensor.matmul(b[:], selrow[:, C * c:C * (c + 1)],
                                 rows_t[0:NCHUNK, :])
                return b

            # ---- LN stats, phase 1: chunk c's channel-sums of x and x^2
            # accumulate into psum partition c of two (7, CW) tiles (selector
            # stationary: column c ones). ----
            def stats_mms(pstatA, pstatB, c, xc_f32, xbc):
                sq = wp.tile([C, CW], bf16, tag="ln_sq")
                nc.scalar.square(sq[:], xc_f32)
                sel = sel1[:, NCHUNK * c:NCHUNK * (c + 1)]
                nc.tensor.matmul(pstatA[0:NCHUNK, :], sel, xbc,
                                 start=(c == 0), stop=(c == NCHUNK - 1))
                nc.tensor.matmul(pstatB[0:NCHUNK, :], sel, sq[:],
                                 start=(c == 0), stop=(c == NCHUNK - 1))

            # ---- LN stats, phase 2 (batched): mean/istd for all chunks ----
            def stats_rows(pstatA, pstatB, u_t, u2_t, var_t, isd_t, eps_t):
                nc.scalar.activation(u2_t[:], pstatA[0:NCHUNK, :], AF.Square,
                                     scale=1.0 / C)
                nc.vector.scalar_tensor_tensor(
                    var_t[:], pstatB[0:NCHUNK, :], 1.0 / C, u2_t[:],
                    OP.mult, OP.subtract)
                sd_t = sp.tile([NCHUNK, CW], f32, tag="sd")
                nc.scalar.activation(sd_t[:], var_t[:], AF.Sqrt,
                                     bias=eps_t[0:NCHUNK, 0:1])
                isdf_t = sp.tile([NCHUNK, CW], f32, tag="isdf")
                nc.vector.reciprocal_approx_fast(out=isdf_t[:], in_=sd_t[:])
                nc.vector.tensor_copy(isd_t[:], isdf_t[:])
                nc.scalar.activation(u_t[:], pstatA[0:NCHUNK, :], AF.Identity,
                                     scale=1.0 / C)

            # ---- LN1 per-chunk normalize + relu + channel-normalize ----
            def ln1_finish(c):
                sl = slice(c * CW, (c + 1) * CW)
                xc = xs[:, sl]
                ub = bcast_row(ln_u, c, name="ubc")
                ib = bcast_row(ln_isd, c, name="ibc")
                xm = wp.tile([C, CW], f32, tag="ln_xm")
                nc.vector.tensor_tensor(xm[:], xc, ub[:], OP.subtract)
                xn = wp.tile([C, CW], f32, tag="ln_xn")
                nc.vector.tensor_tensor(xn[:], xm[:], ib[:], OP.mult)
                rl = wp.tile([C, CW], bf16, tag="ln_rl")
                nc.scalar.activation(rl[:], xn[:], AF.Relu,
                                     bias=ln1b[:, 0:1], scale=ln1w[:, 0:1])
                sb = sumbcast(rl[:], name="lnsb")
                rb = wp.tile([C, CW], f32, tag="ln_rb")
                nc.vector.reciprocal_approx_fast(out=rb[:], in_=sb[:])
                nc.vector.tensor_tensor(xin[:, sl], rl[:], rb[:], OP.mult)

            # ---- NNMF phase builders ----
            def phase_a(c):
                # recon = convT(h): three fp8 DoubleRow matmuls carry the
                # (dy=0, dy=1) pairs, three normal fp8 matmuls the dy=2 row
                y0 = c * CH
                ps = ps_conv.tile([C, CW], f32, tag="conv", name="psA")
                for dx in range(3):
                    lhsT = wr[:, dx * 2 * C:(dx + 1) * 2 * C].rearrange(
                        "p (two m) -> p two m", two=2)
                    rhs = hp4[0:C, 0:2, y0:y0 + CH, dx:dx + W]
                    nc.tensor.matmul(ps[:], lhsT, rhs, start=(dx == 0),
                                     stop=False, perf_mode=DR)
                for dx in range(3):
                    view = hc0[0:C, y0 + 2:y0 + 2 + CH, dx:dx + W]
                    nc.tensor.matmul(ps[:], wr[:, 6 * C + dx * C:
                                                6 * C + (dx + 1) * C], view,
                                     start=False, stop=(dx == 2))
                # recon >= ~1e-5 everywhere (strictly positive weights and h),
                # so the reference's +1e-12 guard is numerically irrelevant
                # and the reciprocal reads PSUM directly.
                rec = wp.tile([C, CW], f32, tag="rec", bufs=6)
                nc.vector.reciprocal_approx_fast(out=rec[:], in_=ps[:])
                ratio_mult(c, rec)

            def ratio_mult(c, rec):
                # ratio = xin * (1/recon), on gpsimd (latency hidden by skew)
                y0 = c * CH
                nc.gpsimd.tensor_tensor(
                    interior(rpad, y0, CH),
                    xin[:, c * CW:(c + 1) * CW], rec[:], OP.mult)

            def phase_b1(c):
                # conv(ratio) and ht = h * conv
                y0 = c * CH
                ps = ps_conv.tile([C, CW], f32, tag="conv", name="psB")
                for k in range(9):
                    dy, dx = k // 3, k % 3
                    view = pad3(rpad)[0:C, y0 + dy:y0 + dy + CH, dx:dx + W]
                    nc.tensor.matmul(ps[:], wc[:, k * C:(k + 1) * C], view,
                                     start=(k == 0), stop=(k == 8))
                ht = wp.tile([C, CW], bf16, tag="ht", bufs=6)
                nc.vector.tensor_tensor(ht[:], hint0(y0, CH), ps[:], OP.mult)
                return ht

            def phase_b2(c, ht):
                # channel sum broadcast to every partition in one matmul
                # (ones 96x96 stationary), then 1/S on the DVE
                sb = sumbcast(ht[:], name="nsb")
                rb = wp.tile([C, CW], f32, tag="nrb", bufs=6)
                nc.vector.reciprocal_approx_fast(out=rb[:], in_=sb[:])
                return rb

            def phase_b3(c, ht, rb, last=False):
                # hpad_new = SH * h_new: ht carries SH*h*conv, rb carries
                # QS/(SH*S), so the stt scalar is SH/QS... both fp8 copies.
                # The LAST iteration's h is only read by the residual add, so
                # it goes to a full-precision f32 tile instead (the fp8
                # round-trip would put ~6% element error straight into the
                # output).
                y0 = c * CH
                if last:
                    nc.vector.scalar_tensor_tensor(
                        hfin[:, c * CW:(c + 1) * CW], ht[:], 1.0 / QS, rb[:],
                        OP.mult, OP.mult)
                    return
                nc.vector.scalar_tensor_tensor(
                    hint0(y0, CH), ht[:], SH / QS, rb[:], OP.mult, OP.mult)
                nc.vector.scalar_tensor_tensor(
                    hint1(y0, CH), ht[:], SH / QS, rb[:], OP.mult, OP.mult)

            # ---- LN2 + MLP + residual ----
            def mlp_p1(pstatA, pstatB, c):
                sl = slice(c * CW, (c + 1) * CW)
                nc.gpsimd.tensor_tensor(x2s[:, sl], xs[:, sl], hfin[:, sl],
                                        OP.add)
                xc = x2s[:, sl]
                x2b = wp.tile([C, CW], bf16, tag="x2b")
                nc.vector.tensor_copy(x2b[:], xc)
                stats_mms(pstatA, pstatB, c, xc, x2b[:])

            def mlp_p2(c):
                sl = slice(c * CW, (c + 1) * CW)
                xc = x2s[:, sl]
                ub = bcast_row(m_u, c, name="ubc")
                ib = bcast_row(m_isd, c, name="ibc")
                xm = wp.tile([C, CW], f32, tag="ln_xm")
                nc.vector.tensor_tensor(xm[:], xc, ub[:], OP.subtract)
                # LN2's affine is folded into w1/b1 on the host, so the
                # normalized value feeds the matmul directly (as bf16).
                xn = wp.tile([C, CW], bf16, tag="ln_xw", bufs=8)
                nc.vector.tensor_tensor(xn[:], xm[:], ib[:], OP.mult)
                return xn

            def mlp_p3(c, xn):
                ys = []
                for j in range(3):
                    p1 = ps_big.tile([128, CW], f32, tag="bcast", name="p1")
                    nc.tensor.matmul(p1[:], w1s[:, j * 128:(j + 1) * 128], xn[:])
                    y1 = wp.tile([128, CW], bf16, tag=f"mlp_y{j}", name=f"mlp_y{j}")
                    if gelu_mode == "hw":
                        nc.scalar.activation(y1[:], p1[:], AF.Gelu,
                                             bias=b1s[:, j:j + 1])
                    else:
                        # CoreSim fallback: sigmoid-GELU (Gelu not implemented
                        # in the simulator). Mirror must match.
                        pre = wp.tile([128, CW], f32, tag=f"mlp_p{j}",
                                      name=f"mlp_p{j}")
                        nc.scalar.activation(pre[:], p1[:], AF.Identity,
                                             bias=b1s[:, j:j + 1])
                        sg = wp.tile([128, CW], f32, tag=f"mlp_s{j}",
                                     name=f"mlp_s{j}")
                        nc.scalar.activation(sg[:], pre[:], AF.Sigmoid,
                                             scale=1.702)
                        nc.vector.tensor_tensor(y1[:], pre[:], sg[:], OP.mult)
                    ys.append(y1)
                return ys

            def mlp_p4(c, ys):
                sl = slice(c * CW, (c + 1) * CW)
                p2 = ps_conv.tile([C, CW], f32, tag="conv")
                for k in range(3):
                    nc.tensor.matmul(p2[:], w2s[k][:], ys[k][:],
                                     start=(k == 0), stop=(k == 2))
                oc = wp.tile([C, CW], f32, tag="oc")
                nc.vector.scalar_tensor_tensor(
                    oc[:], p2[:], b2s[:, 0:1], x2s[:, sl], OP.add, OP.add)
                if c % 2 == 0:
                    nc.sync.dma_start(out_d[:, sl], oc[:])
                else:
                    nc.gpsimd.dma_start(out_d[:, sl], oc[:])

            # ---- LN1 stats phase (prologue, batched rows) ----
            pstat1A = ps_stat.tile([NCHUNK, CW], f32, tag="statA",
                                   name="pstat1A")
            pstat1B = ps_stat.tile([NCHUNK, CW], f32, tag="statB",
                                   name="pstat1B")
            for c in range(NCHUNK):
                sl = slice(c * CW, (c + 1) * CW)
                xbc = wp.tile([C, CW], bf16, tag="x2b")
                nc.vector.tensor_copy(xbc[:], xs[:, sl])
                stats_mms(pstat1A, pstat1B, c, xs[:, sl], xbc[:])
                fillers(3)
            stats_rows(pstat1A, pstat1B, ln_u, ln_u2, ln_var, ln_isd, eps6)
            fillers(6)

            # ---- ONE global software pipeline: LN1-finish chunks play the
            # A-stage role for iteration 0 (its back-projection reciprocal is
            # the precomputed rec0), then every NNMF chunk-slot, then the MLP
            # stages ride the tail. ----
            total = nit * NCHUNK
            hts = {}
            rbs = {}
            xns = {}
            yss = {}
            pstat2A = pstat2B = None
            for s in range(0, total + NCHUNK + 11):
                if s < min(NCHUNK, total):
                    ln1_finish(s)
                    ratio_mult(s, rec0s[:, s * CW:(s + 1) * CW])
                    fillers(5)
                elif s < total:
                    phase_a(s % NCHUNK)
                if total <= s:
                    # MLP tail slots are matmul-sparse; hold the clock open
                    fillers(4)
                c1 = s - 2
                if 0 <= c1 < total:
                    hts[c1] = phase_b1(c1 % NCHUNK)
                c2 = s - 3
                if 0 <= c2 < total:
                    rbs[c2] = phase_b2(c2 % NCHUNK, hts[c2])
                c3 = s - 4
                if 0 <= c3 < total:
                    phase_b3(c3 % NCHUNK, hts.pop(c3), rbs.pop(c3),
                             last=(c3 >= total - NCHUNK))
                # MLP stats (Square on ACT, sums on PE) trail the last
                # iteration's b3 slots; the batched row stage runs once after
                # all 7 chunks' sums are in.
                m1 = s - (total - 2)
                if 0 <= m1 < NCHUNK:
                    if pstat2A is None:
                        pstat2A = ps_stat.tile([NCHUNK, CW], f32,
                                               tag="statA", name="pstat2A")
                        pstat2B = ps_stat.tile([NCHUNK, CW], f32,
                                               tag="statB", name="pstat2B")
                    mlp_p1(pstat2A, pstat2B, m1)
                if m1 == NCHUNK:
                    stats_rows(pstat2A, pstat2B, m_u, m_u2, m_var, m_isd, eps5)
                m2 = s - (total + NCHUNK - 2)
                if 0 <= m2 < NCHUNK:
                    xns[m2] = mlp_p2(m2)
                m3 = s - (total + NCHUNK - 1)
                if 0 <= m3 < NCHUNK:
                    yss[m3] = mlp_p3(m3, xns.pop(m3))
                m4 = s - (total + NCHUNK)
                if 0 <= m4 < NCHUNK:
                    mlp_p4(m4, yss.pop(m4))

    return nc


def _prepare_maps(x, ln1_w, ln1_b, w_nnmf, ln2_w, ln2_b, w1, b1, w2, b2):
    import ml_dtypes
    bf16 = ml_dtypes.bfloat16
    WcD, Wr8, rec0 = _build_conv_mats(w_nnmf)
    f = lambda a: np.ascontiguousarray(np.asarray(a, np.float32))
    fb = lambda a: np.ascontiguousarray(np.asarray(a, np.float32).astype(bf16))
    f8 = lambda a: np.ascontiguousarray(
        np.asarray(a, np.float32).astype(ml_dtypes.float8_e4m3))
    # LN2's per-channel affine folded into the first MLP matmul:
    # (xn*w + b) @ w1 + b1 == xn @ (diag(w) @ w1) + (b1 + b @ w1)
    w1_64 = np.asarray(w1, np.float64)
    w1f = w1_64 * np.asarray(ln2_w, np.float64)[:, None]
    b1f = np.asarray(b1, np.float64) + np.asarray(ln2_b, np.float64) @ w1_64
    # selector stationaries (see _build_bass)
    sel1 = np.zeros((C, NCHUNK * NCHUNK), np.float32)
    selrow = np.zeros((NCHUNK, NCHUNK * C), np.float32)
    for c in range(NCHUNK):
        sel1[:, NCHUNK * c + c] = 1.0
        selrow[c, C * c:C * (c + 1)] = 1.0
    shared = {
        "sel1": fb(sel1),
        "selrow": fb(selrow),
        "rec0": fb(rec0),
        "wrecon": f8(Wr8),
        "wconv": fb(WcD),
        "w1T": fb(w1f),
        "b1": f(b1f).reshape(HID, 1),
        "w2T": fb(w2),
        "b2": f(b2).reshape(C, 1),
        "ln1w": f(ln1_w).reshape(C, 1),
        "ln1b": f(ln1_b).reshape(C, 1),
    }
    xs = np.asarray(x)
    return [dict(shared, x=f(xs[i]).reshape(C, NPIX))
            for i in range(xs.shape[0])]


def kernel(x, ln1_w, ln1_b, w_nnmf, ln2_w, ln2_b, w1, b1, w2, b2):
    global _CACHED_NC, LAST_RESULT
    from concourse.bass_utils import run_bass_kernel_spmd

    if _CACHED_NC is None:
        nc = _build_bass()
        nc.finalize()
        _CACHED_NC = nc
    nc = _CACHED_NC
    in_maps = _prepare_maps(x, ln1_w, ln1_b, w_nnmf, ln2_w, ln2_b, w1, b1, w2, b2)
    res = run_bass_kernel_spmd(nc, in_maps, core_ids=list(range(8)), trace=TRACE)
    LAST_RESULT = res
    out = np.stack([res.results[i]["out"].reshape(C, H, W) for i in range(8)])
    return out.astype(np.float32)
